# revision 10
# baseline (speedup 1.0000x reference)
"""BertSelfAttention (B=4, S=2048, H=1024, NH=16, HD=64) on 8 Trainium2 NeuronCores.

Sharding: batch (4) x head-group (2) -> 8 cores. Core c handles batch b=c//2 and
heads [g*8, g*8+8) with g=c%2 (output channels [g*512, (g+1)*512)).

The kernel is organized so the Scalar engine (which does all softmax exp work,
~285us at 1 elem/cycle/lane) paces the kernel and everything else hides under
it:

  * Scores matmuls are ROW-TILED (K=64): a head pair's two matmuls run
    concurrently in the top/bottom halves of the PE array (tile_position
    (0,0)/(64,0)), so scores cost one N=512 stream per head PAIR instead of
    per head -- 2x over the zero-padded K=128 formulation.
  * Q is packed two heads per [128, S] tile exactly like K (no zero padding).
  * exp runs on [128, 2x512] PSUM windows (2 key tiles per ACT op). The
    attention mask is folded into V and the denominator column as exp(mask)
    per-key scaling, so the ACT op needs no per-key-tile bias and stays wide:
    exp(s/8 + m) = exp(s/8)*exp(m), and both the ctx numerator and the
    denominator contract exp(m) with the keys.
  * ctx keeps the fused denominator row (lhsT = [v' | exp(mask)], M=65,
    K=128); e and v' are bf16 (PSUM accumulation stays fp32).
  * QKV projections are software-pipelined: only K/Q of head-pair 0 run
    before attention starts; V and the remaining pairs' K/Q are emitted as
    PE filler between score windows on 2 dedicated PSUM banks.

PSUM budget (8 banks): score windows [128,2,512] x 2 bufs (4), ctx h0/h1
[65,512] (2), projection ping-pong (2).
"""

import os
import sys
from collections import deque

if "/opt/trn_rl_repo" not in sys.path:
    sys.path.insert(0, "/opt/trn_rl_repo")

import numpy as np

_KERNEL_DIR = os.path.dirname(os.path.abspath(__file__))

B, S, H = 4, 2048, 1024
NH, HD = 16, 64
HPC = 8          # heads per core
CH = HPC * HD    # 512 output channels per core
CT = H // 128    # 8 contraction tiles
NP = 4           # head pairs per core
ST = S // 128    # 16 key tiles
VW = HD + 1      # 65: v columns + fused denominator column
QB = 512         # query block (unit width)
NQ = S // QB     # 4 query blocks

_CACHE = {}


def _build():
    import concourse.bass as bass  # noqa: F401
    import concourse.mybir as mybir
    import concourse.tile as tile
    from concourse import bacc

    F32 = mybir.dt.float32
    F32R = mybir.dt.float32r
    BF16 = mybir.dt.bfloat16
    EXP = mybir.ActivationFunctionType.Exp

    nc = bacc.Bacc("TRN2", target_bir_lowering=False, debug=True)

    xt = nc.dram_tensor("xt", [H, S], F32, kind="ExternalInput")        # x_b^T
    wq_t = nc.dram_tensor("wq_t", [H, CH], F32, kind="ExternalInput")   # wq_c^T
    wk_t = nc.dram_tensor("wk_t", [H, CH], F32, kind="ExternalInput")
    wv_t = nc.dram_tensor("wv_t", [H, CH], F32, kind="ExternalInput")
    bq = nc.dram_tensor("bq", [CH], F32, kind="ExternalInput")
    bk = nc.dram_tensor("bk", [CH], F32, kind="ExternalInput")
    bv = nc.dram_tensor("bv", [CH], F32, kind="ExternalInput")
    mask = nc.dram_tensor("mask", [S], F32, kind="ExternalInput")
    out = nc.dram_tensor("out", [VW * HPC, S], F32, kind="ExternalOutput")

    wq_r = wq_t.rearrange("(c p) j -> c p j", p=128).bitcast(F32R)
    wk_r = wk_t.rearrange("(c p) j -> c p j", p=128).bitcast(F32R)
    wv_r = wv_t.rearrange("(c p) j -> c p j", p=128).bitcast(F32R)
    xt_r = xt.rearrange("(c p) s -> c p s", p=128).bitcast(F32R)

    ADD = mybir.AluOpType.add

    with tile.TileContext(nc) as tc, nc.allow_low_precision(reason="bf16 attn"):
        from contextlib import ExitStack

        with ExitStack() as outer:
            persist = outer.enter_context(tc.tile_pool(name="persist", bufs=1))
            xqp = outer.enter_context(tc.tile_pool(name="xq", bufs=16))
            xvp = outer.enter_context(tc.tile_pool(name="xv", bufs=16))
            vtp = outer.enter_context(tc.tile_pool(name="vt", bufs=3))
            wkqp = outer.enter_context(tc.tile_pool(name="wkq", bufs=16))
            epool = outer.enter_context(tc.tile_pool(name="ep", bufs=20))
            opool = outer.enter_context(tc.tile_pool(name="op", bufs=4))
            ppool = outer.enter_context(tc.tile_pool(name="pp", bufs=1,
                                                     space="PSUM"))
            wpool = outer.enter_context(tc.tile_pool(name="wp", bufs=2,
                                                     space="PSUM"))

            # ---------------- persistent SBUF ----------------
            kt = [persist.tile([128, S], F32R, tag=f"kt{p}", name=f"kt{p}")
                  for p in range(NP)]
            qp = [persist.tile([128, S], F32R, tag=f"qp{p}", name=f"qp{p}")
                  for p in range(NP)]
            v_sb = persist.tile([128, ST, HPC * VW], BF16, tag="v")
            v4 = v_sb.rearrange("p t (h e) -> p t h e", e=VW)
            mask_sb = persist.tile([128, ST], F32, tag="mask")
            em_sb = persist.tile([128, ST], F32, tag="em")
            bq_sb = persist.tile([128, NP], F32, tag="bq")
            bk_sb = persist.tile([128, NP], F32, tag="bk")
            bv_bc = persist.tile([128, CH], F32, tag="bv")
            wv_sb = [persist.tile([128, CH], F32R, tag=f"wv{ct}",
                                  name=f"wv{ct}")
                     for ct in range(CT)]

            nc.sync.dma_start(out=mask_sb,
                              in_=mask.rearrange("(t p) -> p t", p=128))
            nc.sync.dma_start(out=bq_sb,
                              in_=bq.rearrange("(j p) -> p j", p=128))
            nc.sync.dma_start(out=bk_sb,
                              in_=bk.rearrange("(j p) -> p j", p=128))
            nc.sync.dma_start(
                out=bv_bc,
                in_=bass.AP(tensor=bv, offset=0, ap=[[0, 128], [1, CH]]))
            # exp(mask): per-key scaling folded into v' and the ones column
            nc.scalar.activation(em_sb, mask_sb, EXP, bias=0.0, scale=1.0)

            def load_pair_w(p):
                wk_s, wq_s = [], []
                for ct in range(CT):
                    wkt = wkqp.tile([128, 128], F32R, tag="wkq",
                                    name=f"wk{p}_{ct}")
                    nc.sync.dma_start(
                        out=wkt, in_=wk_r[ct, :, p * 128:(p + 1) * 128])
                    wk_s.append(wkt)
                    wqt = wkqp.tile([128, 128], F32R, tag="wkq",
                                    name=f"wq{p}_{ct}")
                    nc.sync.dma_start(
                        out=wqt, in_=wq_r[ct, :, p * 128:(p + 1) * 128])
                    wq_s.append(wqt)
                return wk_s, wq_s

            def drain_kq(psum, dest, pairc, bias_sb, sq):
                nc.vector.tensor_scalar_add(
                    dest[:, sq * QB:(sq + 1) * QB], psum,
                    bias_sb[:, pairc:pairc + 1])

            # ---------------- pre-attention: K/Q of pair 0 ----------------
            # x chunk DMAs run 2 contraction-tiles ahead of the matmuls so
            # the PE never waits on HBM mid-accumulation.
            wk0, wq0 = load_pair_w(0)
            k0t = [wpool.tile([128, 2, QB], F32, tag="w", name=f"k0t{i}")
                   for i in range(2)]
            q0t = [ppool.tile([128, QB], F32, tag=t, name=f"q0t{t}")
                   for t in ("pA", "pB", "cA", "cB")]
            x_pre = {}

            def pre_dma(ct):
                for sq in range(4):
                    x_t = xqp.tile([128, QB], F32R, tag="xq",
                                   name=f"x0_{ct}_{sq}")
                    nc.sync.dma_start(
                        out=x_t, in_=xt_r[ct, :, sq * QB:(sq + 1) * QB])
                    x_pre[(ct, sq)] = x_t

            pre_dma(0)
            pre_dma(1)
            for ct in range(CT):
                if ct == 2:
                    for wct in range(CT):
                        nc.sync.dma_start(out=wv_sb[wct], in_=wv_r[wct])
                if ct + 2 < CT:
                    pre_dma(ct + 2)
                st_, sp_ = (ct == 0), (ct == CT - 1)
                for sq in range(4):
                    x_t = x_pre.pop((ct, sq))
                    nc.tensor.matmul(k0t[sq // 2][:, sq % 2, :],
                                     lhsT=wk0[ct], rhs=x_t,
                                     start=st_, stop=sp_)
                    nc.tensor.matmul(q0t[sq], lhsT=wq0[ct], rhs=x_t,
                                     start=st_, stop=sp_)
            for sq in range(4):
                drain_kq(k0t[sq // 2][:, sq % 2, :], kt[0], 0, bk_sb, sq)
                drain_kq(q0t[sq], qp[0], 0, bq_sb, sq)

            # ---------------- projection fillers ----------------
            # Each fill is ROW-TILED (T0: x rows 0:63, T8: rows 64:127) into
            # the pA/pB partial banks -- the same (64,128) PE config as the
            # score matmuls, so interleaving fills never reconfigures the
            # array. DMA emission leads MM emission by one fill.
            def v_fill(st):
                chunks = []

                def dma():
                    for ct in range(CT):
                        x_t = xvp.tile([128, 128], F32R, tag="xv",
                                       name=f"xv{st}_{ct}")
                        nc.sync.dma_start(
                            out=x_t,
                            in_=xt_r[ct, :, st * 128:(st + 1) * 128])
                        chunks.append(x_t)

                def mms(pa, pb):
                    for ct in range(CT):
                        st_, sp_ = (ct == 0), (ct == CT - 1)
                        nc.tensor.matmul(pa, lhsT=chunks[ct][0:64, :],
                                         rhs=wv_sb[ct][0:64, :],
                                         start=st_, stop=sp_,
                                         tile_position=(0, 0))
                        nc.tensor.matmul(pb, lhsT=chunks[ct][64:128, :],
                                         rhs=wv_sb[ct][64:128, :],
                                         start=st_, stop=sp_,
                                         tile_position=(64, 0))
                    # v' = (pa + pb + bv) * exp(mask); denom col = exp(mask)
                    tmp = vtp.tile([128, CH], F32, tag="vtmp", name=f"vt{st}")
                    nc.vector.tensor_add(tmp, pa, bv_bc)
                    nc.vector.tensor_add(tmp, tmp, pb)
                    nc.vector.tensor_scalar_mul(
                        v4[:, st, :, 0:HD],
                        tmp.rearrange("p (h d) -> p h d", d=HD),
                        em_sb[:, st:st + 1])
                    ems = em_sb[:, st:st + 1]
                    emb = bass.AP(tensor=ems.tensor, offset=ems.offset,
                                  ap=[ems.ap[0], [0, HPC]])
                    nc.vector.tensor_copy(v4[:, st, :, HD], emb)

                return dma, mms

            def kq_fill(kind, p, sq, w_s):
                chunks = []

                def dma():
                    for ct in range(CT):
                        x_t = xqp.tile([128, QB], F32R, tag="xq",
                                       name=f"x{kind}{p}_{ct}_{sq}")
                        nc.sync.dma_start(
                            out=x_t, in_=xt_r[ct, :, sq * QB:(sq + 1) * QB])
                        chunks.append(x_t)

                def mms(pa, pb):
                    for ct in range(CT):
                        st_, sp_ = (ct == 0), (ct == CT - 1)
                        nc.tensor.matmul(pa, lhsT=w_s[ct][0:64, :],
                                         rhs=chunks[ct][0:64, :],
                                         start=st_, stop=sp_,
                                         tile_position=(0, 0))
                        nc.tensor.matmul(pb, lhsT=w_s[ct][64:128, :],
                                         rhs=chunks[ct][64:128, :],
                                         start=st_, stop=sp_,
                                         tile_position=(64, 0))
                    dest = kt[p] if kind == "k" else qp[p]
                    bias = bk_sb if kind == "k" else bq_sb
                    tmp = vtp.tile([128, QB], F32, tag="kqtmp",
                                   name=f"kqt{kind}{p}{sq}")
                    nc.vector.tensor_scalar_add(tmp, pa, bias[:, p:p + 1])
                    nc.vector.tensor_add(
                        dest[:, sq * QB:(sq + 1) * QB], tmp, pb)

                return dma, mms

            proj_fills = [v_fill(st) for st in range(ST)]
            proj_state = {"dma": 0, "mm": 0}

            def emit_proj_fill():
                s = proj_state
                if s["mm"] >= len(proj_fills):
                    return
                while s["dma"] <= s["mm"] + 1 and s["dma"] < len(proj_fills):
                    proj_fills[s["dma"]][0]()
                    s["dma"] += 1
                pa = ppool.tile([128, QB], F32, tag="pA",
                                name=f"prA{s['mm']}")
                pb = ppool.tile([128, QB], F32, tag="pB",
                                name=f"prB{s['mm']}")
                proj_fills[s["mm"]][1](pa, pb)
                s["mm"] += 1

            # proj pacing: unit 0: V fills (16) at 2/window; unit 1: 1/window
            # (pair1 K/Q); units 2-3: every 2nd window; units 4-11: every 4th.
            def fills_for(u, w):
                if u == 0:
                    return 2
                if u == 1:
                    return 1
                if u <= 3:
                    return 1 if w % 2 == 0 else 0
                if u <= 11:
                    return 1 if w % 4 == 0 else 0
                return 0

            # ---------------- main attention loop ----------------
            # Scores/exp stream window-pair by window-pair (ACT-paced); the
            # ctx matmuls of unit u-1 run as one dense 32-MM cluster early in
            # unit u (after 2 window-pairs of scores so ACT has runway).
            units = [(p, c) for p in range(NP) for c in range(NQ)]

            def emit_ctx_cluster(u, p, c, e_tiles):
                banks = [ppool.tile([128, QB], F32, tag=t, name=f"ctx{u}{t}")
                         for t in ("cA", "cB")]
                for h01 in range(2):
                    h = 2 * p + h01
                    for w in range(8):
                        e = e_tiles[w][h01]
                        for j in range(2):
                            g = 2 * w + j
                            nc.tensor.matmul(
                                banks[h01][0:VW, :],
                                lhsT=v4[:, g, h, :],
                                rhs=e[:, j, :],
                                start=(g == 0), stop=(g == ST - 1))
                for h01 in range(2):
                    h = 2 * p + h01
                    o = opool.tile([VW, QB], F32, tag="o", name=f"o{u}_{h01}")
                    nc.vector.tensor_copy(o, banks[h01][0:VW, :])
                    nc.sync.dma_start(
                        out=out[h * VW:(h + 1) * VW, c * QB:(c + 1) * QB],
                        in_=o)

            pair_w = {0: (wk0, wq0)}
            prev_ctx = None
            for u, (p, c) in enumerate(units):
                np_ = {0: 1, 2: 2, 4: 3}.get(u)
                if np_ is not None:
                    pair_w[np_] = load_pair_w(np_)
                    wk_s, wq_s = pair_w[np_]
                    for sq in range(4):
                        proj_fills.append(kq_fill("k", np_, sq, wk_s))
                        proj_fills.append(kq_fill("q", np_, sq, wq_s))

                e_tiles = []
                for w in range(8):
                    wA = wpool.tile([128, 2, QB], F32, tag="w",
                                    name=f"sA{u}_{w}")
                    wB = wpool.tile([128, 2, QB], F32, tag="w",
                                    name=f"sB{u}_{w}")
                    for j in range(2):
                        g = 2 * w + j
                        nc.tensor.matmul(
                            wA[:, j, :],
                            lhsT=kt[p][0:64, g * 128:(g + 1) * 128],
                            rhs=qp[p][0:64, c * QB:(c + 1) * QB],
                            start=True, stop=True, tile_position=(0, 0))
                        nc.tensor.matmul(
                            wB[:, j, :],
                            lhsT=kt[p][64:128, g * 128:(g + 1) * 128],
                            rhs=qp[p][64:128, c * QB:(c + 1) * QB],
                            start=True, stop=True, tile_position=(64, 0))
                    eA = epool.tile([128, 2, QB], BF16, tag="e",
                                    name=f"eA{u}_{w}")
                    nc.scalar.activation(eA, wA, EXP, bias=0.0, scale=0.125)
                    eB = epool.tile([128, 2, QB], BF16, tag="e",
                                    name=f"eB{u}_{w}")
                    nc.scalar.activation(eB, wB, EXP, bias=0.0, scale=0.125)
                    e_tiles.append((eA, eB))

                    if w == 0 and prev_ctx is not None:
                        emit_ctx_cluster(*prev_ctx)
                    for _ in range(fills_for(u, w)):
                        emit_proj_fill()

                prev_ctx = (u, p, c, e_tiles)

            emit_ctx_cluster(*prev_ctx)
            while proj_state["mm"] < len(proj_fills):
                emit_proj_fill()

    nc.compile()
    return nc


def _get_nc():
    if "nc" not in _CACHE:
        _CACHE["nc"] = _build()
    return _CACHE["nc"]


def _in_maps(hidden_states, attention_mask, wq, bq, wk, bk, wv, bv):
    maps = []
    for c in range(8):
        b, g = c // 2, c % 2
        ch0 = g * CH
        maps.append({
            "xt": np.ascontiguousarray(hidden_states[b].T),
            "wq_t": np.ascontiguousarray(wq[ch0:ch0 + CH, :].T),
            "wk_t": np.ascontiguousarray(wk[ch0:ch0 + CH, :].T),
            "wv_t": np.ascontiguousarray(wv[ch0:ch0 + CH, :].T),
            "bq": np.ascontiguousarray(bq[ch0:ch0 + CH]),
            "bk": np.ascontiguousarray(bk[ch0:ch0 + CH]),
            "bv": np.ascontiguousarray(bv[ch0:ch0 + CH]),
            "mask": np.ascontiguousarray(attention_mask[b, 0, 0, :]),
        })
    return maps


def _gather(results):
    full = np.empty((B, S, H), np.float32)
    for c in range(8):
        b, g = c // 2, c % 2
        o = results[c]["out"].reshape(HPC, VW, S)
        ctx = o[:, :HD, :] / o[:, HD:HD + 1, :]
        full[b, :, g * CH:(g + 1) * CH] = ctx.reshape(CH, S).T
    return full


def _run(in_maps, trace=False):
    from concourse.bass_utils import run_bass_kernel_spmd

    nc = _get_nc()
    return run_bass_kernel_spmd(nc, in_maps, list(range(8)), trace=trace)


def _run_results(in_maps):
    """Run on hardware; on a wedged-device error retry in fresh subprocesses."""
    try:
        return _run(in_maps).results
    except Exception:
        pass
    import pickle
    import subprocess
    import tempfile

    last = None
    for _ in range(3):
        try:
            with tempfile.TemporaryDirectory() as td:
                fin = os.path.join(td, "in.pkl")
                fout = os.path.join(td, "out.pkl")
                with open(fin, "wb") as f:
                    pickle.dump(in_maps, f)
                code = (
                    "import pickle, sys\n"
                    f"sys.path.insert(0, {_KERNEL_DIR!r})\n"
                    "import kernel\n"
                    f"maps = pickle.load(open({fin!r}, 'rb'))\n"
                    "res = kernel._run(maps)\n"
                    f"pickle.dump(res.results, open({fout!r}, 'wb'))\n"
                )
                subprocess.run([sys.executable, "-c", code], check=True,
                               timeout=1800)
                with open(fout, "rb") as f:
                    return pickle.load(f)
        except Exception as e:
            last = e
    raise last


def kernel(hidden_states, attention_mask, wq, bq, wk, bk, wv, bv):
    args = [np.asarray(a, np.float32) for a in
            (hidden_states, attention_mask, wq, bq, wk, bk, wv, bv)]
    return _gather(_run_results(_in_maps(*args)))


def kernel_profiled(hidden_states, attention_mask, wq, bq, wk, bk, wv, bv):
    """Like kernel() but with NTFF tracing; returns (output, exec_time_ns)."""
    args = [np.asarray(a, np.float32) for a in
            (hidden_states, attention_mask, wq, bq, wk, bk, wv, bv)]
    res = _run(_in_maps(*args), trace=True)
    return _gather(res.results), res.exec_time_ns


# revision 12
# speedup vs baseline: 1.0368x; 1.0368x over previous
"""BertSelfAttention (B=4, S=2048, H=1024, NH=16, HD=64) on 8 Trainium2 NeuronCores.

Sharding: batch (4) x head-group (2) -> 8 cores. Core c handles batch b=c//2 and
heads [g*8, g*8+8) with g=c%2 (output channels [g*512, (g+1)*512)).

The kernel is organized so the Scalar engine (which does all softmax exp work,
~285us at 1 elem/cycle/lane) paces the kernel and everything else hides under
it:

  * Scores matmuls are ROW-TILED (K=64): a head pair's two matmuls run
    concurrently in the top/bottom halves of the PE array (tile_position
    (0,0)/(64,0)), so scores cost one N=512 stream per head PAIR instead of
    per head -- 2x over the zero-padded K=128 formulation.
  * Q is packed two heads per [128, S] tile exactly like K (no zero padding).
  * exp runs on [128, 2x512] PSUM windows (2 key tiles per ACT op). The
    attention mask is folded into V and the denominator column as exp(mask)
    per-key scaling, so the ACT op needs no per-key-tile bias and stays wide:
    exp(s/8 + m) = exp(s/8)*exp(m), and both the ctx numerator and the
    denominator contract exp(m) with the keys.
  * ctx keeps the fused denominator row (lhsT = [v' | exp(mask)], M=65,
    K=128); e and v' are bf16 (PSUM accumulation stays fp32).
  * QKV projections are software-pipelined: only K/Q of head-pair 0 run
    before attention starts; V and the remaining pairs' K/Q are emitted as
    PE filler between score windows on 2 dedicated PSUM banks.

PSUM budget (8 banks): score windows [128,2,512] x 2 bufs (4), ctx h0/h1
[65,512] (2), projection ping-pong (2).
"""

import os
import sys
from collections import deque

if "/opt/trn_rl_repo" not in sys.path:
    sys.path.insert(0, "/opt/trn_rl_repo")

import numpy as np

_KERNEL_DIR = os.path.dirname(os.path.abspath(__file__))

B, S, H = 4, 2048, 1024
NH, HD = 16, 64
HPC = 8          # heads per core
CH = HPC * HD    # 512 output channels per core
CT = H // 128    # 8 contraction tiles
NP = 4           # head pairs per core
ST = S // 128    # 16 key tiles
VW = HD + 1      # 65: v columns + fused denominator column
QB = 512         # query block (unit width)
NQ = S // QB     # 4 query blocks

_CACHE = {}


def _build():
    import concourse.bass as bass  # noqa: F401
    import concourse.mybir as mybir
    import concourse.tile as tile
    from concourse import bacc

    F32 = mybir.dt.float32
    F32R = mybir.dt.float32r
    BF16 = mybir.dt.bfloat16
    EXP = mybir.ActivationFunctionType.Exp

    nc = bacc.Bacc("TRN2", target_bir_lowering=False, debug=True)

    xt = nc.dram_tensor("xt", [H, S], F32, kind="ExternalInput")        # x_b^T
    wq_t = nc.dram_tensor("wq_t", [H, CH], F32, kind="ExternalInput")   # wq_c^T
    wk_t = nc.dram_tensor("wk_t", [H, CH], F32, kind="ExternalInput")
    wv_t = nc.dram_tensor("wv_t", [H, CH], F32, kind="ExternalInput")
    bq = nc.dram_tensor("bq", [CH], F32, kind="ExternalInput")
    bk = nc.dram_tensor("bk", [CH], F32, kind="ExternalInput")
    bv = nc.dram_tensor("bv", [CH], F32, kind="ExternalInput")
    mask = nc.dram_tensor("mask", [S], F32, kind="ExternalInput")
    out = nc.dram_tensor("out", [VW * HPC, S], F32, kind="ExternalOutput")

    wq_r = wq_t.rearrange("(c p) j -> c p j", p=128).bitcast(F32R)
    wk_r = wk_t.rearrange("(c p) j -> c p j", p=128).bitcast(F32R)
    wv_r = wv_t.rearrange("(c p) j -> c p j", p=128).bitcast(F32R)
    xt_r = xt.rearrange("(c p) s -> c p s", p=128).bitcast(F32R)

    ADD = mybir.AluOpType.add

    with tile.TileContext(nc) as tc, nc.allow_low_precision(reason="bf16 attn"):
        from contextlib import ExitStack

        with ExitStack() as outer:
            persist = outer.enter_context(tc.tile_pool(name="persist", bufs=1))
            xqp = outer.enter_context(tc.tile_pool(name="xq", bufs=16))
            xvp = outer.enter_context(tc.tile_pool(name="xv", bufs=16))
            vtp = outer.enter_context(tc.tile_pool(name="vt", bufs=3))
            wkqp = outer.enter_context(tc.tile_pool(name="wkq", bufs=16))
            epool = outer.enter_context(tc.tile_pool(name="ep", bufs=20))
            opool = outer.enter_context(tc.tile_pool(name="op", bufs=4))
            ppool = outer.enter_context(tc.tile_pool(name="pp", bufs=1,
                                                     space="PSUM"))
            wpool = outer.enter_context(tc.tile_pool(name="wp", bufs=2,
                                                     space="PSUM"))

            # ---------------- persistent SBUF ----------------
            kt = [persist.tile([128, S], F32R, tag=f"kt{p}", name=f"kt{p}")
                  for p in range(NP)]
            qp = [persist.tile([128, S], F32R, tag=f"qp{p}", name=f"qp{p}")
                  for p in range(NP)]
            v_sb = persist.tile([128, ST, HPC * VW], BF16, tag="v")
            v4 = v_sb.rearrange("p t (h e) -> p t h e", e=VW)
            mask_sb = persist.tile([128, ST], F32, tag="mask")
            em_sb = persist.tile([128, ST], F32, tag="em")
            bq_sb = persist.tile([128, NP], F32, tag="bq")
            bk_sb = persist.tile([128, NP], F32, tag="bk")
            bv_bc = persist.tile([128, CH], F32, tag="bv")
            wv_sb = [persist.tile([128, CH], F32R, tag=f"wv{ct}",
                                  name=f"wv{ct}")
                     for ct in range(CT)]

            nc.sync.dma_start(out=mask_sb,
                              in_=mask.rearrange("(t p) -> p t", p=128))
            nc.sync.dma_start(out=bq_sb,
                              in_=bq.rearrange("(j p) -> p j", p=128))
            nc.sync.dma_start(out=bk_sb,
                              in_=bk.rearrange("(j p) -> p j", p=128))
            nc.sync.dma_start(
                out=bv_bc,
                in_=bass.AP(tensor=bv, offset=0, ap=[[0, 128], [1, CH]]))
            # exp(mask): per-key scaling folded into v' and the ones column
            nc.scalar.activation(em_sb, mask_sb, EXP, bias=0.0, scale=1.0)

            def load_pair_w(p):
                wk_s, wq_s = [], []
                for ct in range(CT):
                    wkt = wkqp.tile([128, 128], F32R, tag="wkq",
                                    name=f"wk{p}_{ct}")
                    nc.sync.dma_start(
                        out=wkt, in_=wk_r[ct, :, p * 128:(p + 1) * 128])
                    wk_s.append(wkt)
                    wqt = wkqp.tile([128, 128], F32R, tag="wkq",
                                    name=f"wq{p}_{ct}")
                    nc.sync.dma_start(
                        out=wqt, in_=wq_r[ct, :, p * 128:(p + 1) * 128])
                    wq_s.append(wqt)
                return wk_s, wq_s

            def drain_kq(psum, dest, pairc, bias_sb, sq):
                nc.vector.tensor_scalar_add(
                    dest[:, sq * QB:(sq + 1) * QB], psum,
                    bias_sb[:, pairc:pairc + 1])

            # ---------------- pre-attention: K/Q of pair 0 ----------------
            # x chunk DMAs run 2 contraction-tiles ahead of the matmuls so
            # the PE never waits on HBM mid-accumulation.
            wk0, wq0 = load_pair_w(0)
            k0t = [wpool.tile([128, 2, QB], F32, tag="w", name=f"k0t{i}")
                   for i in range(2)]
            q0t = [ppool.tile([128, QB], F32, tag=t, name=f"q0t{t}")
                   for t in ("pA", "pB", "cA", "cB")]
            x_pre = {}

            def pre_dma(ct):
                for sq in range(4):
                    x_t = xqp.tile([128, QB], F32R, tag="xq",
                                   name=f"x0_{ct}_{sq}")
                    nc.sync.dma_start(
                        out=x_t, in_=xt_r[ct, :, sq * QB:(sq + 1) * QB])
                    x_pre[(ct, sq)] = x_t

            pre_dma(0)
            pre_dma(1)
            for ct in range(CT):
                if ct == 2:
                    for wct in range(CT):
                        nc.sync.dma_start(out=wv_sb[wct], in_=wv_r[wct])
                if ct + 2 < CT:
                    pre_dma(ct + 2)
                st_, sp_ = (ct == 0), (ct == CT - 1)
                for sq in range(4):
                    x_t = x_pre.pop((ct, sq))
                    nc.tensor.matmul(k0t[sq // 2][:, sq % 2, :],
                                     lhsT=wk0[ct], rhs=x_t,
                                     start=st_, stop=sp_)
                    nc.tensor.matmul(q0t[sq], lhsT=wq0[ct], rhs=x_t,
                                     start=st_, stop=sp_)
            for sq in range(4):
                drain_kq(k0t[sq // 2][:, sq % 2, :], kt[0], 0, bk_sb, sq)
                drain_kq(q0t[sq], qp[0], 0, bq_sb, sq)


            # ---------------- projection fillers ----------------
            # Each fill is ROW-TILED (T0: x rows 0:63, T8: rows 64:127) into
            # the pA/pB partial banks -- the same (64,128) PE config as the
            # score matmuls, so interleaving fills never reconfigures the
            # array. DMA emission leads MM emission by one fill.
            # V projection runs pre-attention: full-width K=128 matmuls
            # (weight loads hide behind the N=512 streams), x chunk DMAs two
            # fills ahead, psum rotating over 4 banks.
            vx = {}

            def v_dma(st):
                cl = []
                for ct in range(CT):
                    x_t = xvp.tile([128, 128], F32R, tag="xv",
                                   name=f"xv{st}_{ct}")
                    nc.sync.dma_start(
                        out=x_t, in_=xt_r[ct, :, st * 128:(st + 1) * 128])
                    cl.append(x_t)
                vx[st] = cl

            def v_mms(st, pb):
                cl = vx.pop(st)
                for ct in range(CT):
                    nc.tensor.matmul(pb, lhsT=cl[ct], rhs=wv_sb[ct],
                                     start=(ct == 0), stop=(ct == CT - 1))
                # v' = (psum + bv) * exp(mask); denom col = exp(mask)
                tmp = vtp.tile([128, CH], F32, tag="vtmp", name=f"vt{st}")
                nc.vector.tensor_add(tmp, pb, bv_bc)
                nc.vector.tensor_scalar_mul(
                    v4[:, st, :, 0:HD],
                    tmp.rearrange("p (h d) -> p h d", d=HD),
                    em_sb[:, st:st + 1])
                ems = em_sb[:, st:st + 1]
                emb = bass.AP(tensor=ems.tensor, offset=ems.offset,
                              ap=[ems.ap[0], [0, HPC]])
                nc.vector.tensor_copy(v4[:, st, :, HD], emb)


            # V projection (all 16 token tiles) before attention starts
            v_dma(0)
            v_dma(1)
            vtags = ("pA", "pB", "cA", "cB")
            for st in range(ST):
                if st + 2 < ST:
                    v_dma(st + 2)
                pb = ppool.tile([128, QB], F32, tag=vtags[st % 4],
                                name=f"vps{st}")
                v_mms(st, pb)

            def kq_fill(kind, p, sq, w_s):
                chunks = []

                def dma():
                    for ct in range(CT):
                        x_t = xqp.tile([128, QB], F32R, tag="xq",
                                       name=f"x{kind}{p}_{ct}_{sq}")
                        nc.sync.dma_start(
                            out=x_t, in_=xt_r[ct, :, sq * QB:(sq + 1) * QB])
                        chunks.append(x_t)

                def mms(pb):
                    for ct in range(CT):
                        nc.tensor.matmul(pb, lhsT=w_s[ct], rhs=chunks[ct],
                                         start=(ct == 0), stop=(ct == CT - 1))
                    dest = kt[p] if kind == "k" else qp[p]
                    bias = bk_sb if kind == "k" else bq_sb
                    drain_kq(pb, dest, p, bias, sq)

                return dma, mms

            proj_fills = []
            proj_state = {"dma": 0, "mm": 0, "tog": 0}

            def emit_proj_fill():
                s = proj_state
                if s["mm"] >= len(proj_fills):
                    return
                while s["dma"] <= s["mm"] + 1 and s["dma"] < len(proj_fills):
                    proj_fills[s["dma"]][0]()
                    s["dma"] += 1
                pb = ppool.tile([128, QB], F32, tag=("pA", "pB")[s["tog"]],
                                name=f"pr{s['mm']}")
                s["tog"] ^= 1
                proj_fills[s["mm"]][1](pb)
                s["mm"] += 1

            # K/Q fill pacing (V is done pre-attention): pair1 over units
            # 1-2, pair2 over units 3-6, pair3 over units 7-10.
            def fills_for(u, w):
                if u == 0:
                    return 0
                if u <= 2:
                    return 1 if w % 2 == 0 else 0
                if u <= 10:
                    return 1 if w % 4 == 0 else 0
                return 0

            # ---------------- main attention loop ----------------
            # Scores/exp stream window-pair by window-pair (ACT-paced); the
            # ctx matmuls of unit u-1 run as one dense 32-MM cluster early in
            # unit u (after 2 window-pairs of scores so ACT has runway).
            units = [(p, c) for p in range(NP) for c in range(NQ)]

            def emit_ctx_cluster(u, p, c, e_tiles):
                banks = [ppool.tile([128, QB], F32, tag=t, name=f"ctx{u}{t}")
                         for t in ("cA", "cB")]
                for h01 in range(2):
                    h = 2 * p + h01
                    for w in range(8):
                        e = e_tiles[w][h01]
                        for j in range(2):
                            g = 2 * w + j
                            nc.tensor.matmul(
                                banks[h01][0:VW, :],
                                lhsT=v4[:, g, h, :],
                                rhs=e[:, j, :],
                                start=(g == 0), stop=(g == ST - 1))
                for h01 in range(2):
                    h = 2 * p + h01
                    o = opool.tile([VW, QB], F32, tag="o", name=f"o{u}_{h01}")
                    nc.vector.tensor_copy(o, banks[h01][0:VW, :])
                    nc.sync.dma_start(
                        out=out[h * VW:(h + 1) * VW, c * QB:(c + 1) * QB],
                        in_=o)

            pair_w = {0: (wk0, wq0)}
            prev_ctx = None
            for u, (p, c) in enumerate(units):
                np_ = {0: 1, 3: 2, 7: 3}.get(u)
                if np_ is not None:
                    pair_w[np_] = load_pair_w(np_)
                    wk_s, wq_s = pair_w[np_]
                    for sq in range(4):
                        proj_fills.append(kq_fill("k", np_, sq, wk_s))
                        proj_fills.append(kq_fill("q", np_, sq, wq_s))

                e_tiles = []
                for w in range(8):
                    wA = wpool.tile([128, 2, QB], F32, tag="w",
                                    name=f"sA{u}_{w}")
                    wB = wpool.tile([128, 2, QB], F32, tag="w",
                                    name=f"sB{u}_{w}")
                    for j in range(2):
                        g = 2 * w + j
                        nc.tensor.matmul(
                            wA[:, j, :],
                            lhsT=kt[p][0:64, g * 128:(g + 1) * 128],
                            rhs=qp[p][0:64, c * QB:(c + 1) * QB],
                            start=True, stop=True, tile_position=(0, 0))
                        nc.tensor.matmul(
                            wB[:, j, :],
                            lhsT=kt[p][64:128, g * 128:(g + 1) * 128],
                            rhs=qp[p][64:128, c * QB:(c + 1) * QB],
                            start=True, stop=True, tile_position=(64, 0))
                    eA = epool.tile([128, 2, QB], BF16, tag="e",
                                    name=f"eA{u}_{w}")
                    nc.scalar.activation(eA, wA, EXP, bias=0.0, scale=0.125)
                    eB = epool.tile([128, 2, QB], BF16, tag="e",
                                    name=f"eB{u}_{w}")
                    nc.scalar.activation(eB, wB, EXP, bias=0.0, scale=0.125)
                    e_tiles.append((eA, eB))

                    if w == 0 and prev_ctx is not None:
                        emit_ctx_cluster(*prev_ctx)
                    for _ in range(fills_for(u, w)):
                        emit_proj_fill()

                prev_ctx = (u, p, c, e_tiles)

            emit_ctx_cluster(*prev_ctx)
            while proj_state["mm"] < len(proj_fills):
                emit_proj_fill()

    nc.compile()
    return nc


def _get_nc():
    if "nc" not in _CACHE:
        _CACHE["nc"] = _build()
    return _CACHE["nc"]


def _in_maps(hidden_states, attention_mask, wq, bq, wk, bk, wv, bv):
    maps = []
    for c in range(8):
        b, g = c // 2, c % 2
        ch0 = g * CH
        maps.append({
            "xt": np.ascontiguousarray(hidden_states[b].T),
            "wq_t": np.ascontiguousarray(wq[ch0:ch0 + CH, :].T),
            "wk_t": np.ascontiguousarray(wk[ch0:ch0 + CH, :].T),
            "wv_t": np.ascontiguousarray(wv[ch0:ch0 + CH, :].T),
            "bq": np.ascontiguousarray(bq[ch0:ch0 + CH]),
            "bk": np.ascontiguousarray(bk[ch0:ch0 + CH]),
            "bv": np.ascontiguousarray(bv[ch0:ch0 + CH]),
            "mask": np.ascontiguousarray(attention_mask[b, 0, 0, :]),
        })
    return maps


def _gather(results):
    full = np.empty((B, S, H), np.float32)
    for c in range(8):
        b, g = c // 2, c % 2
        o = results[c]["out"].reshape(HPC, VW, S)
        ctx = o[:, :HD, :] / o[:, HD:HD + 1, :]
        full[b, :, g * CH:(g + 1) * CH] = ctx.reshape(CH, S).T
    return full


def _run(in_maps, trace=False):
    from concourse.bass_utils import run_bass_kernel_spmd

    nc = _get_nc()
    return run_bass_kernel_spmd(nc, in_maps, list(range(8)), trace=trace)


def _run_results(in_maps):
    """Run on hardware; on a wedged-device error retry in fresh subprocesses."""
    try:
        return _run(in_maps).results
    except Exception:
        pass
    import pickle
    import subprocess
    import tempfile

    last = None
    for _ in range(3):
        try:
            with tempfile.TemporaryDirectory() as td:
                fin = os.path.join(td, "in.pkl")
                fout = os.path.join(td, "out.pkl")
                with open(fin, "wb") as f:
                    pickle.dump(in_maps, f)
                code = (
                    "import pickle, sys\n"
                    f"sys.path.insert(0, {_KERNEL_DIR!r})\n"
                    "import kernel\n"
                    f"maps = pickle.load(open({fin!r}, 'rb'))\n"
                    "res = kernel._run(maps)\n"
                    f"pickle.dump(res.results, open({fout!r}, 'wb'))\n"
                )
                subprocess.run([sys.executable, "-c", code], check=True,
                               timeout=1800)
                with open(fout, "rb") as f:
                    return pickle.load(f)
        except Exception as e:
            last = e
    raise last


def kernel(hidden_states, attention_mask, wq, bq, wk, bk, wv, bv):
    args = [np.asarray(a, np.float32) for a in
            (hidden_states, attention_mask, wq, bq, wk, bk, wv, bv)]
    return _gather(_run_results(_in_maps(*args)))


def kernel_profiled(hidden_states, attention_mask, wq, bq, wk, bk, wv, bv):
    """Like kernel() but with NTFF tracing; returns (output, exec_time_ns)."""
    args = [np.asarray(a, np.float32) for a in
            (hidden_states, attention_mask, wq, bq, wk, bk, wv, bv)]
    res = _run(_in_maps(*args), trace=True)
    return _gather(res.results), res.exec_time_ns


# revision 13
# speedup vs baseline: 1.1209x; 1.0812x over previous
"""BertSelfAttention (B=4, S=2048, H=1024, NH=16, HD=64) on 8 Trainium2 NeuronCores.

Sharding: batch (4) x head-group (2) -> 8 cores. Core c handles batch b=c//2 and
heads [g*8, g*8+8) with g=c%2 (output channels [g*512, (g+1)*512)).

The kernel is organized so the Scalar engine (which does all softmax exp work,
~285us at 1 elem/cycle/lane) paces the kernel and everything else hides under
it:

  * Scores matmuls are ROW-TILED (K=64): a head pair's two matmuls run
    concurrently in the top/bottom halves of the PE array (tile_position
    (0,0)/(64,0)), so scores cost one N=512 stream per head PAIR instead of
    per head -- 2x over the zero-padded K=128 formulation.
  * Q is packed two heads per [128, S] tile exactly like K (no zero padding).
  * exp runs on [128, 2x512] PSUM windows (2 key tiles per ACT op). The
    attention mask is folded into V and the denominator column as exp(mask)
    per-key scaling, so the ACT op needs no per-key-tile bias and stays wide:
    exp(s/8 + m) = exp(s/8)*exp(m), and both the ctx numerator and the
    denominator contract exp(m) with the keys.
  * ctx keeps the fused denominator row (lhsT = [v' | exp(mask)], M=65,
    K=128); e and v' are bf16 (PSUM accumulation stays fp32).
  * QKV projections are software-pipelined: only K/Q of head-pair 0 run
    before attention starts; V and the remaining pairs' K/Q are emitted as
    PE filler between score windows on 2 dedicated PSUM banks.

PSUM budget (8 banks): score windows [128,2,512] x 2 bufs (4), ctx h0/h1
[65,512] (2), projection ping-pong (2).
"""

import os
import sys
from collections import deque

if "/opt/trn_rl_repo" not in sys.path:
    sys.path.insert(0, "/opt/trn_rl_repo")

import numpy as np

_KERNEL_DIR = os.path.dirname(os.path.abspath(__file__))

B, S, H = 4, 2048, 1024
NH, HD = 16, 64
HPC = 8          # heads per core
CH = HPC * HD    # 512 output channels per core
CT = H // 128    # 8 contraction tiles
NP = 4           # head pairs per core
ST = S // 128    # 16 key tiles
VW = HD + 1      # 65: v columns + fused denominator column
QB = 512         # query block (unit width)
NQ = S // QB     # 4 query blocks

_CACHE = {}


def _build():
    import concourse.bass as bass  # noqa: F401
    import concourse.mybir as mybir
    import concourse.tile as tile
    from concourse import bacc

    F32 = mybir.dt.float32
    F32R = mybir.dt.float32r
    BF16 = mybir.dt.bfloat16
    EXP = mybir.ActivationFunctionType.Exp

    nc = bacc.Bacc("TRN2", target_bir_lowering=False, debug=True)

    xt = nc.dram_tensor("xt", [H, S], F32, kind="ExternalInput")        # x_b^T
    wq_t = nc.dram_tensor("wq_t", [H, CH], F32, kind="ExternalInput")   # wq_c^T
    wk_t = nc.dram_tensor("wk_t", [H, CH], F32, kind="ExternalInput")
    wv_t = nc.dram_tensor("wv_t", [H, CH], F32, kind="ExternalInput")
    bq = nc.dram_tensor("bq", [CH], F32, kind="ExternalInput")
    bk = nc.dram_tensor("bk", [CH], F32, kind="ExternalInput")
    bv = nc.dram_tensor("bv", [CH], F32, kind="ExternalInput")
    mask = nc.dram_tensor("mask", [S], F32, kind="ExternalInput")
    out = nc.dram_tensor("out", [VW * HPC, S], F32, kind="ExternalOutput")

    wq_r = wq_t.rearrange("(c p) j -> c p j", p=128).bitcast(F32R)
    wk_r = wk_t.rearrange("(c p) j -> c p j", p=128).bitcast(F32R)
    wv_r = wv_t.rearrange("(c p) j -> c p j", p=128).bitcast(F32R)
    xt_r = xt.rearrange("(c p) s -> c p s", p=128).bitcast(F32R)

    ADD = mybir.AluOpType.add

    with tile.TileContext(nc) as tc, nc.allow_low_precision(reason="bf16 attn"):
        from contextlib import ExitStack

        with ExitStack() as outer:
            persist = outer.enter_context(tc.tile_pool(name="persist", bufs=1))
            xqp = outer.enter_context(tc.tile_pool(name="xq", bufs=16))
            xvp = outer.enter_context(tc.tile_pool(name="xv", bufs=16))
            vtp = outer.enter_context(tc.tile_pool(name="vt", bufs=3))
            wkqp = outer.enter_context(tc.tile_pool(name="wkq", bufs=16))
            epool = outer.enter_context(tc.tile_pool(name="ep", bufs=12))
            opool = outer.enter_context(tc.tile_pool(name="op", bufs=4))
            ppool = outer.enter_context(tc.tile_pool(name="pp", bufs=1,
                                                     space="PSUM"))
            wpool = outer.enter_context(tc.tile_pool(name="wp", bufs=2,
                                                     space="PSUM"))

            # ---------------- persistent SBUF ----------------
            kt = [persist.tile([128, S], BF16, tag=f"kt{p}", name=f"kt{p}")
                  for p in range(NP)]
            qp = [persist.tile([128, S], BF16, tag=f"qp{p}", name=f"qp{p}")
                  for p in range(NP)]
            v_sb = persist.tile([128, ST, HPC * VW], BF16, tag="v")
            v4 = v_sb.rearrange("p t (h e) -> p t h e", e=VW)
            mask_sb = persist.tile([128, ST], F32, tag="mask")
            em_sb = persist.tile([128, ST], F32, tag="em")
            bq_sb = persist.tile([128, NP], F32, tag="bq")
            bk_sb = persist.tile([128, NP], F32, tag="bk")
            bv_bc = persist.tile([128, CH], F32, tag="bv")
            wv_sb = [persist.tile([128, CH], F32R, tag=f"wv{ct}",
                                  name=f"wv{ct}")
                     for ct in range(CT)]

            nc.sync.dma_start(out=mask_sb,
                              in_=mask.rearrange("(t p) -> p t", p=128))
            nc.sync.dma_start(out=bq_sb,
                              in_=bq.rearrange("(j p) -> p j", p=128))
            nc.sync.dma_start(out=bk_sb,
                              in_=bk.rearrange("(j p) -> p j", p=128))
            nc.sync.dma_start(
                out=bv_bc,
                in_=bass.AP(tensor=bv, offset=0, ap=[[0, 128], [1, CH]]))
            # exp(mask): per-key scaling folded into v' and the ones column
            nc.scalar.activation(em_sb, mask_sb, EXP, bias=0.0, scale=1.0)

            def load_pair_w(p):
                wk_s, wq_s = [], []
                for ct in range(CT):
                    wkt = wkqp.tile([128, 128], F32R, tag="wkq",
                                    name=f"wk{p}_{ct}")
                    nc.sync.dma_start(
                        out=wkt, in_=wk_r[ct, :, p * 128:(p + 1) * 128])
                    wk_s.append(wkt)
                    wqt = wkqp.tile([128, 128], F32R, tag="wkq",
                                    name=f"wq{p}_{ct}")
                    nc.sync.dma_start(
                        out=wqt, in_=wq_r[ct, :, p * 128:(p + 1) * 128])
                    wq_s.append(wqt)
                return wk_s, wq_s

            def drain_kq(psum, dest, pairc, bias_sb, sq):
                nc.vector.tensor_scalar_add(
                    dest[:, sq * QB:(sq + 1) * QB], psum,
                    bias_sb[:, pairc:pairc + 1])

            # ---------------- pre-attention: K/Q of pair 0 ----------------
            # x chunk DMAs run 2 contraction-tiles ahead of the matmuls so
            # the PE never waits on HBM mid-accumulation.
            wk0, wq0 = load_pair_w(0)
            k0t = [wpool.tile([128, 2, QB], F32, tag="w", name=f"k0t{i}")
                   for i in range(2)]
            q0t = [ppool.tile([128, QB], F32, tag=t, name=f"q0t{t}")
                   for t in ("pA", "pB", "cA", "cB")]
            x_pre = {}

            def pre_dma(ct):
                for sq in range(4):
                    x_t = xqp.tile([128, QB], F32R, tag="xq",
                                   name=f"x0_{ct}_{sq}")
                    nc.sync.dma_start(
                        out=x_t, in_=xt_r[ct, :, sq * QB:(sq + 1) * QB])
                    x_pre[(ct, sq)] = x_t

            pre_dma(0)
            pre_dma(1)
            for ct in range(CT):
                if ct == 2:
                    for wct in range(CT):
                        nc.sync.dma_start(out=wv_sb[wct], in_=wv_r[wct])
                if ct + 2 < CT:
                    pre_dma(ct + 2)
                st_, sp_ = (ct == 0), (ct == CT - 1)
                for sq in range(4):
                    x_t = x_pre.pop((ct, sq))
                    nc.tensor.matmul(k0t[sq // 2][:, sq % 2, :],
                                     lhsT=wk0[ct], rhs=x_t,
                                     start=st_, stop=sp_)
                    nc.tensor.matmul(q0t[sq], lhsT=wq0[ct], rhs=x_t,
                                     start=st_, stop=sp_)
            for sq in range(4):
                drain_kq(k0t[sq // 2][:, sq % 2, :], kt[0], 0, bk_sb, sq)
                drain_kq(q0t[sq], qp[0], 0, bq_sb, sq)


            # ---------------- projection fillers ----------------
            # Each fill is ROW-TILED (T0: x rows 0:63, T8: rows 64:127) into
            # the pA/pB partial banks -- the same (64,128) PE config as the
            # score matmuls, so interleaving fills never reconfigures the
            # array. DMA emission leads MM emission by one fill.
            # V projection runs pre-attention: full-width K=128 matmuls
            # (weight loads hide behind the N=512 streams), x chunk DMAs two
            # fills ahead, psum rotating over 4 banks.
            vx = {}

            def v_dma(st):
                cl = []
                for ct in range(CT):
                    x_t = xvp.tile([128, 128], F32R, tag="xv",
                                   name=f"xv{st}_{ct}")
                    nc.sync.dma_start(
                        out=x_t, in_=xt_r[ct, :, st * 128:(st + 1) * 128])
                    cl.append(x_t)
                vx[st] = cl

            def v_mms(st, pb):
                cl = vx.pop(st)
                for ct in range(CT):
                    nc.tensor.matmul(pb, lhsT=cl[ct], rhs=wv_sb[ct],
                                     start=(ct == 0), stop=(ct == CT - 1))
                # v' = (psum + bv) * exp(mask); denom col = exp(mask)
                tmp = vtp.tile([128, CH], F32, tag="vtmp", name=f"vt{st}")
                nc.vector.tensor_add(tmp, pb, bv_bc)
                nc.vector.tensor_scalar_mul(
                    v4[:, st, :, 0:HD],
                    tmp.rearrange("p (h d) -> p h d", d=HD),
                    em_sb[:, st:st + 1])
                ems = em_sb[:, st:st + 1]
                emb = bass.AP(tensor=ems.tensor, offset=ems.offset,
                              ap=[ems.ap[0], [0, HPC]])
                nc.vector.tensor_copy(v4[:, st, :, HD], emb)


            # V projection (all 16 token tiles) before attention starts
            v_dma(0)
            v_dma(1)
            vtags = ("pA", "pB", "cA", "cB")
            for st in range(ST):
                if st + 2 < ST:
                    v_dma(st + 2)
                pb = ppool.tile([128, QB], F32, tag=vtags[st % 4],
                                name=f"vps{st}")
                v_mms(st, pb)

            def kq_fill(kind, p, sq, w_s):
                chunks = []

                def dma():
                    for ct in range(CT):
                        x_t = xqp.tile([128, QB], F32R, tag="xq",
                                       name=f"x{kind}{p}_{ct}_{sq}")
                        nc.sync.dma_start(
                            out=x_t, in_=xt_r[ct, :, sq * QB:(sq + 1) * QB])
                        chunks.append(x_t)

                def mms(pb):
                    for ct in range(CT):
                        nc.tensor.matmul(pb, lhsT=w_s[ct], rhs=chunks[ct],
                                         start=(ct == 0), stop=(ct == CT - 1))
                    dest = kt[p] if kind == "k" else qp[p]
                    bias = bk_sb if kind == "k" else bq_sb
                    drain_kq(pb, dest, p, bias, sq)

                return dma, mms

            proj_fills = []
            proj_state = {"dma": 0, "mm": 0, "tog": 0}

            def emit_proj_fill():
                s = proj_state
                if s["mm"] >= len(proj_fills):
                    return
                while s["dma"] <= s["mm"] + 1 and s["dma"] < len(proj_fills):
                    proj_fills[s["dma"]][0]()
                    s["dma"] += 1
                pb = ppool.tile([128, QB], F32, tag=("pA", "pB")[s["tog"]],
                                name=f"pr{s['mm']}")
                s["tog"] ^= 1
                proj_fills[s["mm"]][1](pb)
                s["mm"] += 1

            # K/Q fill pacing (V is done pre-attention): pair1 over units
            # 1-2, pair2 over units 3-6, pair3 over units 7-10.
            def fills_for(u, w):
                if u == 0:
                    return 0
                if u <= 2:
                    return 1 if w % 2 == 0 else 0
                if u <= 10:
                    return 1 if w % 4 == 0 else 0
                return 0

            # ---------------- main attention loop ----------------
            # Scores/exp stream window-pair by window-pair (ACT-paced); the
            # ctx matmuls of unit u-1 run as one dense 32-MM cluster early in
            # unit u (after 2 window-pairs of scores so ACT has runway).
            units = [(p, c) for p in range(NP) for c in range(NQ)]
            ctx_fifo = deque()
            ctx_banks = {}

            def emit_ctx_window(item):
                u, p, c, w, eA, eB = item
                if w == 0:
                    ctx_banks[u] = [
                        ppool.tile([128, QB], F32, tag=t, name=f"ctx{u}{t}")
                        for t in ("cA", "cB")]
                for h01, e in ((0, eA), (1, eB)):
                    h = 2 * p + h01
                    cb = ctx_banks[u][h01]
                    for j in range(2):
                        g = 2 * w + j
                        nc.tensor.matmul(
                            cb[0:VW, :],
                            lhsT=v4[:, g, h, :],
                            rhs=e[:, j, :],
                            start=(g == 0), stop=(g == ST - 1))
                if w == 7:
                    for h01 in range(2):
                        h = 2 * p + h01
                        o = opool.tile([VW, QB], F32, tag="o",
                                       name=f"o{u}_{h01}")
                        nc.vector.tensor_copy(o, ctx_banks[u][h01][0:VW, :])
                        nc.sync.dma_start(
                            out=out[h * VW:(h + 1) * VW,
                                    c * QB:(c + 1) * QB],
                            in_=o)
                    del ctx_banks[u]

            pair_w = {0: (wk0, wq0)}
            for u, (p, c) in enumerate(units):
                np_ = {0: 1, 3: 2, 7: 3}.get(u)
                if np_ is not None:
                    pair_w[np_] = load_pair_w(np_)
                    wk_s, wq_s = pair_w[np_]
                    for sq in range(4):
                        proj_fills.append(kq_fill("k", np_, sq, wk_s))
                        proj_fills.append(kq_fill("q", np_, sq, wq_s))

                last_unit = (u == len(units) - 1)
                for w in range(8):
                    wA = wpool.tile([128, 2, QB], F32, tag="w",
                                    name=f"sA{u}_{w}")
                    wB = wpool.tile([128, 2, QB], F32, tag="w",
                                    name=f"sB{u}_{w}")
                    for j in range(2):
                        g = 2 * w + j
                        nc.tensor.matmul(
                            wA[:, j, :],
                            lhsT=kt[p][0:64, g * 128:(g + 1) * 128],
                            rhs=qp[p][0:64, c * QB:(c + 1) * QB],
                            start=True, stop=True, tile_position=(0, 0))
                    eA = epool.tile([128, 2, QB], BF16, tag="e",
                                    name=f"eA{u}_{w}")
                    nc.scalar.activation(eA, wA, EXP, bias=0.0, scale=0.125)
                    for j in range(2):
                        g = 2 * w + j
                        nc.tensor.matmul(
                            wB[:, j, :],
                            lhsT=kt[p][64:128, g * 128:(g + 1) * 128],
                            rhs=qp[p][64:128, c * QB:(c + 1) * QB],
                            start=True, stop=True, tile_position=(64, 0))
                    eB = epool.tile([128, 2, QB], BF16, tag="e",
                                    name=f"eB{u}_{w}")
                    nc.scalar.activation(eB, wB, EXP, bias=0.0, scale=0.125)
                    ctx_fifo.append((u, p, c, w, eA, eB))

                    lag = 0 if last_unit else 3
                    while len(ctx_fifo) > lag:
                        emit_ctx_window(ctx_fifo.popleft())
                    for _ in range(fills_for(u, w)):
                        emit_proj_fill()

            while ctx_fifo:
                emit_ctx_window(ctx_fifo.popleft())
            while proj_state["mm"] < len(proj_fills):
                emit_proj_fill()

    nc.compile()
    return nc


def _get_nc():
    if "nc" not in _CACHE:
        _CACHE["nc"] = _build()
    return _CACHE["nc"]


def _in_maps(hidden_states, attention_mask, wq, bq, wk, bk, wv, bv):
    maps = []
    for c in range(8):
        b, g = c // 2, c % 2
        ch0 = g * CH
        maps.append({
            "xt": np.ascontiguousarray(hidden_states[b].T),
            "wq_t": np.ascontiguousarray(wq[ch0:ch0 + CH, :].T),
            "wk_t": np.ascontiguousarray(wk[ch0:ch0 + CH, :].T),
            "wv_t": np.ascontiguousarray(wv[ch0:ch0 + CH, :].T),
            "bq": np.ascontiguousarray(bq[ch0:ch0 + CH]),
            "bk": np.ascontiguousarray(bk[ch0:ch0 + CH]),
            "bv": np.ascontiguousarray(bv[ch0:ch0 + CH]),
            "mask": np.ascontiguousarray(attention_mask[b, 0, 0, :]),
        })
    return maps


def _gather(results):
    full = np.empty((B, S, H), np.float32)
    for c in range(8):
        b, g = c // 2, c % 2
        o = results[c]["out"].reshape(HPC, VW, S)
        ctx = o[:, :HD, :] / o[:, HD:HD + 1, :]
        full[b, :, g * CH:(g + 1) * CH] = ctx.reshape(CH, S).T
    return full


def _run(in_maps, trace=False):
    from concourse.bass_utils import run_bass_kernel_spmd

    nc = _get_nc()
    return run_bass_kernel_spmd(nc, in_maps, list(range(8)), trace=trace)


def _run_results(in_maps):
    """Run on hardware; on a wedged-device error retry in fresh subprocesses."""
    try:
        return _run(in_maps).results
    except Exception:
        pass
    import pickle
    import subprocess
    import tempfile

    last = None
    for _ in range(3):
        try:
            with tempfile.TemporaryDirectory() as td:
                fin = os.path.join(td, "in.pkl")
                fout = os.path.join(td, "out.pkl")
                with open(fin, "wb") as f:
                    pickle.dump(in_maps, f)
                code = (
                    "import pickle, sys\n"
                    f"sys.path.insert(0, {_KERNEL_DIR!r})\n"
                    "import kernel\n"
                    f"maps = pickle.load(open({fin!r}, 'rb'))\n"
                    "res = kernel._run(maps)\n"
                    f"pickle.dump(res.results, open({fout!r}, 'wb'))\n"
                )
                subprocess.run([sys.executable, "-c", code], check=True,
                               timeout=1800)
                with open(fout, "rb") as f:
                    return pickle.load(f)
        except Exception as e:
            last = e
    raise last


def kernel(hidden_states, attention_mask, wq, bq, wk, bk, wv, bv):
    args = [np.asarray(a, np.float32) for a in
            (hidden_states, attention_mask, wq, bq, wk, bk, wv, bv)]
    return _gather(_run_results(_in_maps(*args)))


def kernel_profiled(hidden_states, attention_mask, wq, bq, wk, bk, wv, bv):
    """Like kernel() but with NTFF tracing; returns (output, exec_time_ns)."""
    args = [np.asarray(a, np.float32) for a in
            (hidden_states, attention_mask, wq, bq, wk, bk, wv, bv)]
    res = _run(_in_maps(*args), trace=True)
    return _gather(res.results), res.exec_time_ns


# revision 14
# speedup vs baseline: 1.1534x; 1.0290x over previous
"""BertSelfAttention (B=4, S=2048, H=1024, NH=16, HD=64) on 8 Trainium2 NeuronCores.

Sharding: batch (4) x head-group (2) -> 8 cores. Core c handles batch b=c//2 and
heads [g*8, g*8+8) with g=c%2 (output channels [g*512, (g+1)*512)).

The kernel is organized so the Scalar engine (which does all softmax exp work,
~285us at 1 elem/cycle/lane) paces the kernel and everything else hides under
it:

  * Scores matmuls are ROW-TILED (K=64): a head pair's two matmuls run
    concurrently in the top/bottom halves of the PE array (tile_position
    (0,0)/(64,0)), so scores cost one N=512 stream per head PAIR instead of
    per head -- 2x over the zero-padded K=128 formulation.
  * Q is packed two heads per [128, S] tile exactly like K (no zero padding).
  * exp runs on [128, 2x512] PSUM windows (2 key tiles per ACT op). The
    attention mask is folded into V and the denominator column as exp(mask)
    per-key scaling, so the ACT op needs no per-key-tile bias and stays wide:
    exp(s/8 + m) = exp(s/8)*exp(m), and both the ctx numerator and the
    denominator contract exp(m) with the keys.
  * ctx keeps the fused denominator row (lhsT = [v' | exp(mask)], M=65,
    K=128); e and v' are bf16 (PSUM accumulation stays fp32).
  * QKV projections are software-pipelined: only K/Q of head-pair 0 run
    before attention starts; V and the remaining pairs' K/Q are emitted as
    PE filler between score windows on 2 dedicated PSUM banks.

PSUM budget (8 banks): score windows [128,2,512] x 2 bufs (4), ctx h0/h1
[65,512] (2), projection ping-pong (2).
"""

import os
import sys
from collections import deque

if "/opt/trn_rl_repo" not in sys.path:
    sys.path.insert(0, "/opt/trn_rl_repo")

import numpy as np

_KERNEL_DIR = os.path.dirname(os.path.abspath(__file__))

B, S, H = 4, 2048, 1024
NH, HD = 16, 64
HPC = 8          # heads per core
CH = HPC * HD    # 512 output channels per core
CT = H // 128    # 8 contraction tiles
NP = 4           # head pairs per core
ST = S // 128    # 16 key tiles
VW = HD + 1      # 65: v columns + fused denominator column
QB = 512         # query block (unit width)
NQ = S // QB     # 4 query blocks

_CACHE = {}


def _build():
    import concourse.bass as bass  # noqa: F401
    import concourse.mybir as mybir
    import concourse.tile as tile
    from concourse import bacc

    F32 = mybir.dt.float32
    F32R = mybir.dt.float32r
    BF16 = mybir.dt.bfloat16
    EXP = mybir.ActivationFunctionType.Exp

    nc = bacc.Bacc("TRN2", target_bir_lowering=False, debug=True)

    xt = nc.dram_tensor("xt", [H, S], F32, kind="ExternalInput")        # x_b^T
    wq_t = nc.dram_tensor("wq_t", [H, CH], F32, kind="ExternalInput")   # wq_c^T
    wk_t = nc.dram_tensor("wk_t", [H, CH], F32, kind="ExternalInput")
    wv_t = nc.dram_tensor("wv_t", [H, CH], F32, kind="ExternalInput")
    bq = nc.dram_tensor("bq", [CH], F32, kind="ExternalInput")
    bk = nc.dram_tensor("bk", [CH], F32, kind="ExternalInput")
    bv = nc.dram_tensor("bv", [CH], F32, kind="ExternalInput")
    mask = nc.dram_tensor("mask", [S], F32, kind="ExternalInput")
    out = nc.dram_tensor("out", [VW * HPC, S], F32, kind="ExternalOutput")

    wq_r = wq_t.rearrange("(c p) j -> c p j", p=128).bitcast(F32R)
    wk_r = wk_t.rearrange("(c p) j -> c p j", p=128).bitcast(F32R)
    wv_r = wv_t.rearrange("(c p) j -> c p j", p=128).bitcast(F32R)
    xt_r = xt.rearrange("(c p) s -> c p s", p=128).bitcast(F32R)

    ADD = mybir.AluOpType.add

    with tile.TileContext(nc) as tc, nc.allow_low_precision(reason="bf16 attn"):
        from contextlib import ExitStack

        with ExitStack() as outer:
            persist = outer.enter_context(tc.tile_pool(name="persist", bufs=1))
            xqp = outer.enter_context(tc.tile_pool(name="xq", bufs=24))
            xvp = outer.enter_context(tc.tile_pool(name="xv", bufs=24))
            vtp = outer.enter_context(tc.tile_pool(name="vt", bufs=3))
            wkqp = outer.enter_context(tc.tile_pool(name="wkq", bufs=16))
            epool = outer.enter_context(tc.tile_pool(name="ep", bufs=22))
            opool = outer.enter_context(tc.tile_pool(name="op", bufs=4))
            ppool = outer.enter_context(tc.tile_pool(name="pp", bufs=1,
                                                     space="PSUM"))
            wpool = outer.enter_context(tc.tile_pool(name="wp", bufs=2,
                                                     space="PSUM"))

            # ---------------- persistent SBUF ----------------
            kt = [persist.tile([128, S], BF16, tag=f"kt{p}", name=f"kt{p}")
                  for p in range(NP)]
            qp = [persist.tile([128, S], BF16, tag=f"qp{p}", name=f"qp{p}")
                  for p in range(NP)]
            v_sb = persist.tile([128, ST, HPC * VW], BF16, tag="v")
            v4 = v_sb.rearrange("p t (h e) -> p t h e", e=VW)
            mask_sb = persist.tile([128, ST], F32, tag="mask")
            em_sb = persist.tile([128, ST], F32, tag="em")
            bq_sb = persist.tile([128, NP], F32, tag="bq")
            bk_sb = persist.tile([128, NP], F32, tag="bk")
            bv_bc = persist.tile([128, CH], F32, tag="bv")
            wv_sb = [persist.tile([128, CH], F32R, tag=f"wv{ct}",
                                  name=f"wv{ct}")
                     for ct in range(CT)]

            nc.sync.dma_start(out=mask_sb,
                              in_=mask.rearrange("(t p) -> p t", p=128))
            nc.sync.dma_start(out=bq_sb,
                              in_=bq.rearrange("(j p) -> p j", p=128))
            nc.sync.dma_start(out=bk_sb,
                              in_=bk.rearrange("(j p) -> p j", p=128))
            nc.sync.dma_start(
                out=bv_bc,
                in_=bass.AP(tensor=bv, offset=0, ap=[[0, 128], [1, CH]]))
            # exp(mask): per-key scaling folded into v' and the ones column
            nc.scalar.activation(em_sb, mask_sb, EXP, bias=0.0, scale=1.0)

            def load_pair_w(p):
                wk_s, wq_s = [], []
                for ct in range(CT):
                    wkt = wkqp.tile([128, 128], F32R, tag="wkq",
                                    name=f"wk{p}_{ct}")
                    nc.sync.dma_start(
                        out=wkt, in_=wk_r[ct, :, p * 128:(p + 1) * 128])
                    wk_s.append(wkt)
                    wqt = wkqp.tile([128, 128], F32R, tag="wkq",
                                    name=f"wq{p}_{ct}")
                    nc.sync.dma_start(
                        out=wqt, in_=wq_r[ct, :, p * 128:(p + 1) * 128])
                    wq_s.append(wqt)
                return wk_s, wq_s

            def drain_kq(psum, dest, pairc, bias_sb, sq):
                nc.vector.tensor_scalar_add(
                    dest[:, sq * QB:(sq + 1) * QB], psum,
                    bias_sb[:, pairc:pairc + 1])

            # ---------------- pre-attention: K/Q of pair 0 ----------------
            # x chunk DMAs run 2 contraction-tiles ahead of the matmuls so
            # the PE never waits on HBM mid-accumulation.
            wk0, wq0 = load_pair_w(0)
            k0t = [wpool.tile([128, 2, QB], F32, tag="w", name=f"k0t{i}")
                   for i in range(2)]
            q0t = [ppool.tile([128, QB], F32, tag=t, name=f"q0t{t}")
                   for t in ("pA", "pB", "cA", "cB")]
            x_pre = {}

            def pre_dma(ct):
                for sq in range(4):
                    x_t = xqp.tile([128, QB], F32R, tag="xq",
                                   name=f"x0_{ct}_{sq}")
                    nc.sync.dma_start(
                        out=x_t, in_=xt_r[ct, :, sq * QB:(sq + 1) * QB])
                    x_pre[(ct, sq)] = x_t

            pre_dma(0)
            pre_dma(1)
            for ct in range(CT):
                if ct == 2:
                    for wct in range(CT):
                        nc.sync.dma_start(out=wv_sb[wct], in_=wv_r[wct])
                if ct + 2 < CT:
                    pre_dma(ct + 2)
                st_, sp_ = (ct == 0), (ct == CT - 1)
                for sq in range(4):
                    x_t = x_pre.pop((ct, sq))
                    nc.tensor.matmul(k0t[sq // 2][:, sq % 2, :],
                                     lhsT=wk0[ct], rhs=x_t,
                                     start=st_, stop=sp_)
                    nc.tensor.matmul(q0t[sq], lhsT=wq0[ct], rhs=x_t,
                                     start=st_, stop=sp_)
            for sq in range(4):
                drain_kq(k0t[sq // 2][:, sq % 2, :], kt[0], 0, bk_sb, sq)
                drain_kq(q0t[sq], qp[0], 0, bq_sb, sq)


            # ---------------- projection fillers ----------------
            # Each fill is ROW-TILED (T0: x rows 0:63, T8: rows 64:127) into
            # the pA/pB partial banks -- the same (64,128) PE config as the
            # score matmuls, so interleaving fills never reconfigures the
            # array. DMA emission leads MM emission by one fill.
            # V projection runs pre-attention: full-width K=128 matmuls
            # (weight loads hide behind the N=512 streams), x chunk DMAs two
            # fills ahead, psum rotating over 4 banks.
            vx = {}

            def v_dma(st):
                cl = []
                for ct in range(CT):
                    x_t = xvp.tile([128, 128], F32R, tag="xv",
                                   name=f"xv{st}_{ct}")
                    nc.sync.dma_start(
                        out=x_t, in_=xt_r[ct, :, st * 128:(st + 1) * 128])
                    cl.append(x_t)
                vx[st] = cl

            def v_mms(st, pb):
                cl = vx.pop(st)
                for ct in range(CT):
                    nc.tensor.matmul(pb, lhsT=cl[ct], rhs=wv_sb[ct],
                                     start=(ct == 0), stop=(ct == CT - 1))
                # v' = (psum + bv) * exp(mask); denom col = exp(mask)
                tmp = vtp.tile([128, CH], F32, tag="vtmp", name=f"vt{st}")
                nc.vector.tensor_add(tmp, pb, bv_bc)
                nc.vector.tensor_scalar_mul(
                    v4[:, st, :, 0:HD],
                    tmp.rearrange("p (h d) -> p h d", d=HD),
                    em_sb[:, st:st + 1])
                ems = em_sb[:, st:st + 1]
                emb = bass.AP(tensor=ems.tensor, offset=ems.offset,
                              ap=[ems.ap[0], [0, HPC]])
                nc.vector.tensor_copy(v4[:, st, :, HD], emb)



            def kq_fill(p, sq, wk_s, wq_s):
                # one fill projects BOTH K and Q of (pair, sq) from a single
                # set of x chunks (halves the x re-stream traffic)
                chunks = []

                def dma():
                    for ct in range(CT):
                        x_t = xqp.tile([128, QB], F32R, tag="xq",
                                       name=f"xkq{p}_{ct}_{sq}")
                        nc.sync.dma_start(
                            out=x_t, in_=xt_r[ct, :, sq * QB:(sq + 1) * QB])
                        chunks.append(x_t)

                def mms():
                    pk = ppool.tile([128, QB], F32, tag="pA",
                                    name=f"prk{p}{sq}")
                    pq = ppool.tile([128, QB], F32, tag="pB",
                                    name=f"prq{p}{sq}")
                    for ct in range(CT):
                        nc.tensor.matmul(pk, lhsT=wk_s[ct], rhs=chunks[ct],
                                         start=(ct == 0), stop=(ct == CT - 1))
                    for ct in range(CT):
                        nc.tensor.matmul(pq, lhsT=wq_s[ct], rhs=chunks[ct],
                                         start=(ct == 0), stop=(ct == CT - 1))
                    drain_kq(pk, kt[p], p, bk_sb, sq)
                    drain_kq(pq, qp[p], p, bq_sb, sq)

                return dma, mms

            def v_fill(st):
                def dma():
                    v_dma(st)

                def mms():
                    s = proj_state
                    pb = ppool.tile([128, QB], F32,
                                    tag=("pA", "pB")[s["tog"]],
                                    name=f"prv{st}")
                    s["tog"] ^= 1
                    v_mms(st, pb)

                return dma, mms

            proj_fills = [v_fill(st) for st in range(ST)]
            proj_state = {"dma": 0, "mm": 0, "tog": 0}

            def emit_proj_fill():
                s = proj_state
                if s["mm"] >= len(proj_fills):
                    return
                while s["dma"] <= s["mm"] + 2 and s["dma"] < len(proj_fills):
                    proj_fills[s["dma"]][0]()
                    s["dma"] += 1
                proj_fills[s["mm"]][1]()
                s["mm"] += 1

            # fill pacing: unit 0 carries V (16 fills at 2/window); merged
            # K+Q fills: pair1 over units 1-3, pair2 4-7, pair3 8-11.
            def fills_for(u, w):
                if u == 0:
                    return 2
                if u <= 3:
                    return 1 if w in (2, 6) else 0
                if u <= 11:
                    return 1 if w == 3 else 0
                return 0

            # ---------------- main attention loop ----------------
            # Scores/exp stream window-pair by window-pair (ACT-paced); the
            # ctx matmuls of unit u-1 run as one dense 32-MM cluster early in
            # unit u (after 2 window-pairs of scores so ACT has runway).
            units = [(p, c) for p in range(NP) for c in range(NQ)]
            ctx_fifo = deque()
            ctx_banks = {}

            def emit_ctx_window(item):
                u, p, c, w, eA, eB = item
                if w == 0:
                    ctx_banks[u] = [
                        ppool.tile([128, QB], F32, tag=t, name=f"ctx{u}{t}")
                        for t in ("cA", "cB")]
                for h01, e in ((0, eA), (1, eB)):
                    h = 2 * p + h01
                    cb = ctx_banks[u][h01]
                    for j in range(2):
                        g = 2 * w + j
                        nc.tensor.matmul(
                            cb[0:VW, :],
                            lhsT=v4[:, g, h, :],
                            rhs=e[:, j, :],
                            start=(g == 0), stop=(g == ST - 1))
                if w == 7:
                    for h01 in range(2):
                        h = 2 * p + h01
                        o = opool.tile([VW, QB], F32, tag="o",
                                       name=f"o{u}_{h01}")
                        nc.vector.tensor_copy(o, ctx_banks[u][h01][0:VW, :])
                        nc.sync.dma_start(
                            out=out[h * VW:(h + 1) * VW,
                                    c * QB:(c + 1) * QB],
                            in_=o)
                    del ctx_banks[u]

            pair_w = {0: (wk0, wq0)}
            for u, (p, c) in enumerate(units):
                np_ = {0: 1, 3: 2, 7: 3}.get(u)
                if np_ is not None:
                    pair_w[np_] = load_pair_w(np_)
                    wk_s, wq_s = pair_w[np_]
                    for sq in range(4):
                        proj_fills.append(kq_fill(np_, sq, wk_s, wq_s))

                last_unit = (u == len(units) - 1)
                for w in range(8):
                    wA = wpool.tile([128, 2, QB], F32, tag="w",
                                    name=f"sA{u}_{w}")
                    wB = wpool.tile([128, 2, QB], F32, tag="w",
                                    name=f"sB{u}_{w}")
                    for j in range(2):
                        g = 2 * w + j
                        nc.tensor.matmul(
                            wA[:, j, :],
                            lhsT=kt[p][0:64, g * 128:(g + 1) * 128],
                            rhs=qp[p][0:64, c * QB:(c + 1) * QB],
                            start=True, stop=True, tile_position=(0, 0))
                    eA = epool.tile([128, 2, QB], BF16, tag="e",
                                    name=f"eA{u}_{w}")
                    nc.scalar.activation(eA, wA, EXP, bias=0.0, scale=0.125)
                    for j in range(2):
                        g = 2 * w + j
                        nc.tensor.matmul(
                            wB[:, j, :],
                            lhsT=kt[p][64:128, g * 128:(g + 1) * 128],
                            rhs=qp[p][64:128, c * QB:(c + 1) * QB],
                            start=True, stop=True, tile_position=(64, 0))
                    eB = epool.tile([128, 2, QB], BF16, tag="e",
                                    name=f"eB{u}_{w}")
                    nc.scalar.activation(eB, wB, EXP, bias=0.0, scale=0.125)
                    ctx_fifo.append((u, p, c, w, eA, eB))

                    if last_unit:
                        lag, max_pop = 0, 99
                    elif u == 0:
                        lag, max_pop = 16, 0
                    else:
                        lag, max_pop = 3, 2
                    pops = 0
                    while len(ctx_fifo) > lag and pops < max_pop:
                        emit_ctx_window(ctx_fifo.popleft())
                        pops += 1
                    for _ in range(fills_for(u, w)):
                        emit_proj_fill()

            while ctx_fifo:
                emit_ctx_window(ctx_fifo.popleft())
            while proj_state["mm"] < len(proj_fills):
                emit_proj_fill()

    nc.compile()
    return nc


def _get_nc():
    if "nc" not in _CACHE:
        _CACHE["nc"] = _build()
    return _CACHE["nc"]


def _in_maps(hidden_states, attention_mask, wq, bq, wk, bk, wv, bv):
    maps = []
    for c in range(8):
        b, g = c // 2, c % 2
        ch0 = g * CH
        maps.append({
            "xt": np.ascontiguousarray(hidden_states[b].T),
            "wq_t": np.ascontiguousarray(wq[ch0:ch0 + CH, :].T),
            "wk_t": np.ascontiguousarray(wk[ch0:ch0 + CH, :].T),
            "wv_t": np.ascontiguousarray(wv[ch0:ch0 + CH, :].T),
            "bq": np.ascontiguousarray(bq[ch0:ch0 + CH]),
            "bk": np.ascontiguousarray(bk[ch0:ch0 + CH]),
            "bv": np.ascontiguousarray(bv[ch0:ch0 + CH]),
            "mask": np.ascontiguousarray(attention_mask[b, 0, 0, :]),
        })
    return maps


def _gather(results):
    full = np.empty((B, S, H), np.float32)
    for c in range(8):
        b, g = c // 2, c % 2
        o = results[c]["out"].reshape(HPC, VW, S)
        ctx = o[:, :HD, :] / o[:, HD:HD + 1, :]
        full[b, :, g * CH:(g + 1) * CH] = ctx.reshape(CH, S).T
    return full


def _run(in_maps, trace=False):
    from concourse.bass_utils import run_bass_kernel_spmd

    nc = _get_nc()
    return run_bass_kernel_spmd(nc, in_maps, list(range(8)), trace=trace)


def _run_results(in_maps):
    """Run on hardware; on a wedged-device error retry in fresh subprocesses."""
    try:
        return _run(in_maps).results
    except Exception:
        pass
    import pickle
    import subprocess
    import tempfile

    last = None
    for _ in range(3):
        try:
            with tempfile.TemporaryDirectory() as td:
                fin = os.path.join(td, "in.pkl")
                fout = os.path.join(td, "out.pkl")
                with open(fin, "wb") as f:
                    pickle.dump(in_maps, f)
                code = (
                    "import pickle, sys\n"
                    f"sys.path.insert(0, {_KERNEL_DIR!r})\n"
                    "import kernel\n"
                    f"maps = pickle.load(open({fin!r}, 'rb'))\n"
                    "res = kernel._run(maps)\n"
                    f"pickle.dump(res.results, open({fout!r}, 'wb'))\n"
                )
                subprocess.run([sys.executable, "-c", code], check=True,
                               timeout=1800)
                with open(fout, "rb") as f:
                    return pickle.load(f)
        except Exception as e:
            last = e
    raise last


def kernel(hidden_states, attention_mask, wq, bq, wk, bk, wv, bv):
    args = [np.asarray(a, np.float32) for a in
            (hidden_states, attention_mask, wq, bq, wk, bk, wv, bv)]
    return _gather(_run_results(_in_maps(*args)))


def kernel_profiled(hidden_states, attention_mask, wq, bq, wk, bk, wv, bv):
    """Like kernel() but with NTFF tracing; returns (output, exec_time_ns)."""
    args = [np.asarray(a, np.float32) for a in
            (hidden_states, attention_mask, wq, bq, wk, bk, wv, bv)]
    res = _run(_in_maps(*args), trace=True)
    return _gather(res.results), res.exec_time_ns


# revision 16
# speedup vs baseline: 1.1757x; 1.0193x over previous
"""BertSelfAttention (B=4, S=2048, H=1024, NH=16, HD=64) on 8 Trainium2 NeuronCores.

Sharding: batch (4) x head-group (2) -> 8 cores. Core c handles batch b=c//2 and
heads [g*8, g*8+8) with g=c%2 (output channels [g*512, (g+1)*512)).

The kernel is organized so the Scalar engine (which does all softmax exp work,
~285us at 1 elem/cycle/lane) paces the kernel and everything else hides under
it:

  * Scores matmuls are ROW-TILED (K=64): a head pair's two matmuls run
    concurrently in the top/bottom halves of the PE array (tile_position
    (0,0)/(64,0)), so scores cost one N=512 stream per head PAIR instead of
    per head -- 2x over the zero-padded K=128 formulation.
  * Q is packed two heads per [128, S] tile exactly like K (no zero padding).
  * exp runs on [128, 2x512] PSUM windows (2 key tiles per ACT op). The
    attention mask is folded into V and the denominator column as exp(mask)
    per-key scaling, so the ACT op needs no per-key-tile bias and stays wide:
    exp(s/8 + m) = exp(s/8)*exp(m), and both the ctx numerator and the
    denominator contract exp(m) with the keys.
  * ctx keeps the fused denominator row (lhsT = [v' | exp(mask)], M=65,
    K=128); e and v' are bf16 (PSUM accumulation stays fp32).
  * QKV projections are software-pipelined: only K/Q of head-pair 0 run
    before attention starts; V and the remaining pairs' K/Q are emitted as
    PE filler between score windows on 2 dedicated PSUM banks.

PSUM budget (8 banks): score windows [128,2,512] x 2 bufs (4), ctx h0/h1
[65,512] (2), projection ping-pong (2).
"""

import os
import sys
from collections import deque

if "/opt/trn_rl_repo" not in sys.path:
    sys.path.insert(0, "/opt/trn_rl_repo")

import numpy as np

_KERNEL_DIR = os.path.dirname(os.path.abspath(__file__))

B, S, H = 4, 2048, 1024
NH, HD = 16, 64
HPC = 8          # heads per core
CH = HPC * HD    # 512 output channels per core
CT = H // 128    # 8 contraction tiles
NP = 4           # head pairs per core
ST = S // 128    # 16 key tiles
VW = HD + 1      # 65: v columns + fused denominator column
QB = 512         # query block (unit width)
NQ = S // QB     # 4 query blocks

_CACHE = {}


def _build():
    import concourse.bass as bass  # noqa: F401
    import concourse.mybir as mybir
    import concourse.tile as tile
    from concourse import bacc

    F32 = mybir.dt.float32
    F32R = mybir.dt.float32r
    BF16 = mybir.dt.bfloat16
    EXP = mybir.ActivationFunctionType.Exp

    nc = bacc.Bacc("TRN2", target_bir_lowering=False, debug=True)

    xt = nc.dram_tensor("xt", [H, S], F32, kind="ExternalInput")        # x_b^T
    wq_t = nc.dram_tensor("wq_t", [H, CH], F32, kind="ExternalInput")   # wq_c^T
    wk_t = nc.dram_tensor("wk_t", [H, CH], F32, kind="ExternalInput")
    wv_t = nc.dram_tensor("wv_t", [H, CH], F32, kind="ExternalInput")
    bq = nc.dram_tensor("bq", [CH], F32, kind="ExternalInput")
    bk = nc.dram_tensor("bk", [CH], F32, kind="ExternalInput")
    bv = nc.dram_tensor("bv", [CH], F32, kind="ExternalInput")
    mask = nc.dram_tensor("mask", [S], F32, kind="ExternalInput")
    out = nc.dram_tensor("out", [VW * HPC, S], F32, kind="ExternalOutput")

    wq_r = wq_t.rearrange("(c p) j -> c p j", p=128).bitcast(F32R)
    wk_r = wk_t.rearrange("(c p) j -> c p j", p=128).bitcast(F32R)
    wv_r = wv_t.rearrange("(c p) j -> c p j", p=128).bitcast(F32R)
    xt_r = xt.rearrange("(c p) s -> c p s", p=128).bitcast(F32R)

    ADD = mybir.AluOpType.add

    with tile.TileContext(nc) as tc, nc.allow_low_precision(reason="bf16 attn"):
        from contextlib import ExitStack

        with ExitStack() as outer:
            persist = outer.enter_context(tc.tile_pool(name="persist", bufs=1))
            xqp = outer.enter_context(tc.tile_pool(name="xq", bufs=24))
            xvp = outer.enter_context(tc.tile_pool(name="xv", bufs=24))
            vtp = outer.enter_context(tc.tile_pool(name="vt", bufs=3))
            wkqp = outer.enter_context(tc.tile_pool(name="wkq", bufs=16))
            epool = outer.enter_context(tc.tile_pool(name="ep", bufs=22))
            opool = outer.enter_context(tc.tile_pool(name="op", bufs=4))
            ppool = outer.enter_context(tc.tile_pool(name="pp", bufs=1,
                                                     space="PSUM"))
            wpool = outer.enter_context(tc.tile_pool(name="wp", bufs=2,
                                                     space="PSUM"))

            # ---------------- persistent SBUF ----------------
            kt = [persist.tile([128, S], BF16, tag=f"kt{p}", name=f"kt{p}")
                  for p in range(NP)]
            qp = [persist.tile([128, S], BF16, tag=f"qp{p}", name=f"qp{p}")
                  for p in range(NP)]
            v_sb = persist.tile([128, ST, HPC * VW], BF16, tag="v")
            v4 = v_sb.rearrange("p t (h e) -> p t h e", e=VW)
            mask_sb = persist.tile([128, ST], F32, tag="mask")
            em_sb = persist.tile([128, ST], F32, tag="em")
            bq_sb = persist.tile([128, NP], F32, tag="bq")
            bk_sb = persist.tile([128, NP], F32, tag="bk")
            bv_bc = persist.tile([128, CH], F32, tag="bv")
            wv_sb = [persist.tile([128, CH], F32R, tag=f"wv{ct}",
                                  name=f"wv{ct}")
                     for ct in range(CT)]

            nc.sync.dma_start(out=mask_sb,
                              in_=mask.rearrange("(t p) -> p t", p=128))
            nc.sync.dma_start(out=bq_sb,
                              in_=bq.rearrange("(j p) -> p j", p=128))
            nc.sync.dma_start(out=bk_sb,
                              in_=bk.rearrange("(j p) -> p j", p=128))
            nc.sync.dma_start(
                out=bv_bc,
                in_=bass.AP(tensor=bv, offset=0, ap=[[0, 128], [1, CH]]))
            # exp(mask): per-key scaling folded into v' and the ones column
            nc.scalar.activation(em_sb, mask_sb, EXP, bias=0.0, scale=1.0)

            def load_pair_w(p):
                wk_s, wq_s = [], []
                for ct in range(CT):
                    wkt = wkqp.tile([128, 128], F32R, tag="wkq",
                                    name=f"wk{p}_{ct}")
                    nc.sync.dma_start(
                        out=wkt, in_=wk_r[ct, :, p * 128:(p + 1) * 128])
                    wk_s.append(wkt)
                    wqt = wkqp.tile([128, 128], F32R, tag="wkq",
                                    name=f"wq{p}_{ct}")
                    nc.sync.dma_start(
                        out=wqt, in_=wq_r[ct, :, p * 128:(p + 1) * 128])
                    wq_s.append(wqt)
                return wk_s, wq_s

            def drain_kq(psum, dest, pairc, bias_sb, sq):
                nc.vector.tensor_scalar_add(
                    dest[:, sq * QB:(sq + 1) * QB], psum,
                    bias_sb[:, pairc:pairc + 1])

            # ---------------- pre-attention: K/Q of pair 0 ----------------
            # x chunk DMAs run 2 contraction-tiles ahead of the matmuls so
            # the PE never waits on HBM mid-accumulation.
            wk0, wq0 = load_pair_w(0)
            k0t = [wpool.tile([128, 2, QB], F32, tag="w", name=f"k0t{i}")
                   for i in range(2)]
            q0t = [ppool.tile([128, QB], F32, tag=t, name=f"q0t{t}")
                   for t in ("pA", "pB", "cA", "cB")]
            x_pre = {}

            def pre_dma(ct):
                for sq in range(4):
                    x_t = xqp.tile([128, QB], F32R, tag="xq",
                                   name=f"x0_{ct}_{sq}")
                    nc.sync.dma_start(
                        out=x_t, in_=xt_r[ct, :, sq * QB:(sq + 1) * QB])
                    x_pre[(ct, sq)] = x_t

            for c0 in range(4):
                pre_dma(c0)
            for ct in range(CT):
                if ct == 2:
                    for wct in range(CT):
                        nc.sync.dma_start(out=wv_sb[wct], in_=wv_r[wct])
                if ct + 4 < CT:
                    pre_dma(ct + 4)
                st_, sp_ = (ct == 0), (ct == CT - 1)
                for sq in range(4):
                    x_t = x_pre.pop((ct, sq))
                    nc.tensor.matmul(k0t[sq // 2][:, sq % 2, :],
                                     lhsT=wk0[ct], rhs=x_t,
                                     start=st_, stop=sp_)
                    nc.tensor.matmul(q0t[sq], lhsT=wq0[ct], rhs=x_t,
                                     start=st_, stop=sp_)
            for sq in range(4):
                drain_kq(k0t[sq // 2][:, sq % 2, :], kt[0], 0, bk_sb, sq)
                drain_kq(q0t[sq], qp[0], 0, bq_sb, sq)


            # ---------------- projection fillers ----------------
            # Each fill is ROW-TILED (T0: x rows 0:63, T8: rows 64:127) into
            # the pA/pB partial banks -- the same (64,128) PE config as the
            # score matmuls, so interleaving fills never reconfigures the
            # array. DMA emission leads MM emission by one fill.
            # V projection runs pre-attention: full-width K=128 matmuls
            # (weight loads hide behind the N=512 streams), x chunk DMAs two
            # fills ahead, psum rotating over 4 banks.
            vx = {}

            def v_dma(st):
                cl = []
                for ct in range(CT):
                    x_t = xvp.tile([128, 128], F32R, tag="xv",
                                   name=f"xv{st}_{ct}")
                    nc.sync.dma_start(
                        out=x_t, in_=xt_r[ct, :, st * 128:(st + 1) * 128])
                    cl.append(x_t)
                vx[st] = cl

            def v_mms(st, pb):
                cl = vx.pop(st)
                for ct in range(CT):
                    nc.tensor.matmul(pb, lhsT=cl[ct], rhs=wv_sb[ct],
                                     start=(ct == 0), stop=(ct == CT - 1))
                # v' = (psum + bv) * exp(mask); denom col = exp(mask)
                tmp = vtp.tile([128, CH], F32, tag="vtmp", name=f"vt{st}")
                nc.vector.tensor_add(tmp, pb, bv_bc)
                nc.vector.tensor_scalar_mul(
                    v4[:, st, :, 0:HD],
                    tmp.rearrange("p (h d) -> p h d", d=HD),
                    em_sb[:, st:st + 1])
                ems = em_sb[:, st:st + 1]
                emb = bass.AP(tensor=ems.tensor, offset=ems.offset,
                              ap=[ems.ap[0], [0, HPC]])
                nc.vector.tensor_copy(v4[:, st, :, HD], emb)



            def kq_fill(p, sq, wk_s, wq_s):
                # one fill projects BOTH K and Q of (pair, sq) from a single
                # set of x chunks (halves the x re-stream traffic)
                chunks = []

                def dma():
                    for ct in range(CT):
                        x_t = xqp.tile([128, QB], F32R, tag="xq",
                                       name=f"xkq{p}_{ct}_{sq}")
                        nc.sync.dma_start(
                            out=x_t, in_=xt_r[ct, :, sq * QB:(sq + 1) * QB])
                        chunks.append(x_t)

                def mms():
                    pk = ppool.tile([128, QB], F32, tag="pA",
                                    name=f"prk{p}{sq}")
                    pq = ppool.tile([128, QB], F32, tag="pB",
                                    name=f"prq{p}{sq}")
                    for ct in range(CT):
                        nc.tensor.matmul(pk, lhsT=wk_s[ct], rhs=chunks[ct],
                                         start=(ct == 0), stop=(ct == CT - 1))
                    for ct in range(CT):
                        nc.tensor.matmul(pq, lhsT=wq_s[ct], rhs=chunks[ct],
                                         start=(ct == 0), stop=(ct == CT - 1))
                    drain_kq(pk, kt[p], p, bk_sb, sq)
                    drain_kq(pq, qp[p], p, bq_sb, sq)

                return dma, mms

            def v_fill(st):
                def dma():
                    v_dma(st)

                def mms():
                    s = proj_state
                    pb = ppool.tile([128, QB], F32,
                                    tag=("pA", "pB")[s["tog"]],
                                    name=f"prv{st}")
                    s["tog"] ^= 1
                    v_mms(st, pb)

                return dma, mms

            proj_fills = [v_fill(st) for st in range(ST)]
            proj_state = {"dma": 0, "mm": 0, "tog": 0}

            def emit_proj_fill():
                s = proj_state
                if s["mm"] >= len(proj_fills):
                    return
                while s["dma"] <= s["mm"] + 2 and s["dma"] < len(proj_fills):
                    proj_fills[s["dma"]][0]()
                    s["dma"] += 1
                proj_fills[s["mm"]][1]()
                s["mm"] += 1

            # fill pacing: unit 0 carries V (16 fills at 2/window); merged
            # K+Q fills: pair1 over units 1-3, pair2 4-7, pair3 8-11.
            def fills_for(u, w):
                if u == 0:
                    return 1
                if u <= 3:
                    return 1 if w in (2, 6) else 0
                if u <= 11:
                    return 1 if w == 3 else 0
                return 0

            # V st0-7 immediately (overlaps the pre-attention DMA tail)
            for _ in range(8):
                emit_proj_fill()

            # ---------------- main attention loop ----------------
            # Scores/exp stream window-pair by window-pair (ACT-paced); the
            # ctx matmuls of unit u-1 run as one dense 32-MM cluster early in
            # unit u (after 2 window-pairs of scores so ACT has runway).
            units = [(p, c) for p in range(NP) for c in range(NQ)]
            ctx_fifo = deque()
            ctx_banks = {}

            def emit_ctx_window(item):
                u, p, c, w, eA, eB = item
                if w == 0:
                    ctx_banks[u] = [
                        ppool.tile([128, QB], F32, tag=t, name=f"ctx{u}{t}")
                        for t in ("cA", "cB")]
                for h01, e in ((0, eA), (1, eB)):
                    h = 2 * p + h01
                    cb = ctx_banks[u][h01]
                    for j in range(2):
                        g = 2 * w + j
                        nc.tensor.matmul(
                            cb[0:VW, :],
                            lhsT=v4[:, g, h, :],
                            rhs=e[:, j, :],
                            start=(g == 0), stop=(g == ST - 1))
                if w == 7:
                    for h01 in range(2):
                        h = 2 * p + h01
                        o = opool.tile([VW, QB], F32, tag="o",
                                       name=f"o{u}_{h01}")
                        nc.vector.tensor_copy(o, ctx_banks[u][h01][0:VW, :])
                        nc.sync.dma_start(
                            out=out[h * VW:(h + 1) * VW,
                                    c * QB:(c + 1) * QB],
                            in_=o)
                    del ctx_banks[u]

            pair_w = {0: (wk0, wq0)}
            for u, (p, c) in enumerate(units):
                np_ = {0: 1, 3: 2, 7: 3}.get(u)
                if np_ is not None:
                    pair_w[np_] = load_pair_w(np_)
                    wk_s, wq_s = pair_w[np_]
                    for sq in range(4):
                        proj_fills.append(kq_fill(np_, sq, wk_s, wq_s))

                last_unit = (u == len(units) - 1)
                for w in range(8):
                    wA = wpool.tile([128, 2, QB], F32, tag="w",
                                    name=f"sA{u}_{w}")
                    wB = wpool.tile([128, 2, QB], F32, tag="w",
                                    name=f"sB{u}_{w}")
                    for j in range(2):
                        g = 2 * w + j
                        nc.tensor.matmul(
                            wA[:, j, :],
                            lhsT=kt[p][0:64, g * 128:(g + 1) * 128],
                            rhs=qp[p][0:64, c * QB:(c + 1) * QB],
                            start=True, stop=True, tile_position=(0, 0))
                    eA = epool.tile([128, 2, QB], BF16, tag="e",
                                    name=f"eA{u}_{w}")
                    nc.scalar.activation(eA, wA, EXP, bias=0.0, scale=0.125)
                    for j in range(2):
                        g = 2 * w + j
                        nc.tensor.matmul(
                            wB[:, j, :],
                            lhsT=kt[p][64:128, g * 128:(g + 1) * 128],
                            rhs=qp[p][64:128, c * QB:(c + 1) * QB],
                            start=True, stop=True, tile_position=(64, 0))
                    eB = epool.tile([128, 2, QB], BF16, tag="e",
                                    name=f"eB{u}_{w}")
                    nc.scalar.activation(eB, wB, EXP, bias=0.0, scale=0.125)
                    ctx_fifo.append((u, p, c, w, eA, eB))

                    if last_unit:
                        lag, max_pop = 0, 99
                    elif u == 0:
                        lag, max_pop = 16, 0
                    else:
                        lag, max_pop = 3, 2
                    pops = 0
                    while len(ctx_fifo) > lag and pops < max_pop:
                        emit_ctx_window(ctx_fifo.popleft())
                        pops += 1
                    for _ in range(fills_for(u, w)):
                        emit_proj_fill()

            while ctx_fifo:
                emit_ctx_window(ctx_fifo.popleft())
            while proj_state["mm"] < len(proj_fills):
                emit_proj_fill()

    nc.compile()
    return nc


def _get_nc():
    if "nc" not in _CACHE:
        _CACHE["nc"] = _build()
    return _CACHE["nc"]


def _in_maps(hidden_states, attention_mask, wq, bq, wk, bk, wv, bv):
    maps = []
    for c in range(8):
        b, g = c // 2, c % 2
        ch0 = g * CH
        maps.append({
            "xt": np.ascontiguousarray(hidden_states[b].T),
            "wq_t": np.ascontiguousarray(wq[ch0:ch0 + CH, :].T),
            "wk_t": np.ascontiguousarray(wk[ch0:ch0 + CH, :].T),
            "wv_t": np.ascontiguousarray(wv[ch0:ch0 + CH, :].T),
            "bq": np.ascontiguousarray(bq[ch0:ch0 + CH]),
            "bk": np.ascontiguousarray(bk[ch0:ch0 + CH]),
            "bv": np.ascontiguousarray(bv[ch0:ch0 + CH]),
            "mask": np.ascontiguousarray(attention_mask[b, 0, 0, :]),
        })
    return maps


def _gather(results):
    full = np.empty((B, S, H), np.float32)
    for c in range(8):
        b, g = c // 2, c % 2
        o = results[c]["out"].reshape(HPC, VW, S)
        ctx = o[:, :HD, :] / o[:, HD:HD + 1, :]
        full[b, :, g * CH:(g + 1) * CH] = ctx.reshape(CH, S).T
    return full


def _run(in_maps, trace=False):
    from concourse.bass_utils import run_bass_kernel_spmd

    nc = _get_nc()
    return run_bass_kernel_spmd(nc, in_maps, list(range(8)), trace=trace)


def _run_results(in_maps):
    """Run on hardware; on a wedged-device error retry in fresh subprocesses."""
    try:
        return _run(in_maps).results
    except Exception:
        pass
    import pickle
    import subprocess
    import tempfile

    last = None
    for _ in range(3):
        try:
            with tempfile.TemporaryDirectory() as td:
                fin = os.path.join(td, "in.pkl")
                fout = os.path.join(td, "out.pkl")
                with open(fin, "wb") as f:
                    pickle.dump(in_maps, f)
                code = (
                    "import pickle, sys\n"
                    f"sys.path.insert(0, {_KERNEL_DIR!r})\n"
                    "import kernel\n"
                    f"maps = pickle.load(open({fin!r}, 'rb'))\n"
                    "res = kernel._run(maps)\n"
                    f"pickle.dump(res.results, open({fout!r}, 'wb'))\n"
                )
                subprocess.run([sys.executable, "-c", code], check=True,
                               timeout=1800)
                with open(fout, "rb") as f:
                    return pickle.load(f)
        except Exception as e:
            last = e
    raise last


def kernel(hidden_states, attention_mask, wq, bq, wk, bk, wv, bv):
    args = [np.asarray(a, np.float32) for a in
            (hidden_states, attention_mask, wq, bq, wk, bk, wv, bv)]
    return _gather(_run_results(_in_maps(*args)))


def kernel_profiled(hidden_states, attention_mask, wq, bq, wk, bk, wv, bv):
    """Like kernel() but with NTFF tracing; returns (output, exec_time_ns)."""
    args = [np.asarray(a, np.float32) for a in
            (hidden_states, attention_mask, wq, bq, wk, bk, wv, bv)]
    res = _run(_in_maps(*args), trace=True)
    return _gather(res.results), res.exec_time_ns


# revision 17
# speedup vs baseline: 1.1808x; 1.0043x over previous
"""BertSelfAttention (B=4, S=2048, H=1024, NH=16, HD=64) on 8 Trainium2 NeuronCores.

Sharding: batch (4) x head-group (2) -> 8 cores. Core c handles batch b=c//2 and
heads [g*8, g*8+8) with g=c%2 (output channels [g*512, (g+1)*512)).

Per-core math (all on device):
  QT[ch, s] = (wq_c @ x_b^T + bq_c),  KT likewise       (channels on partitions)
  V[s, ch]  = (x_b @ wv_c^T + bv_c)                     (tokens on partitions)
  per (head h, query half ih), per key tile st (128 keys j):
      scoresT[j, i] -> [128, 1024] PSUM (2 matmuls), ping-pong buffered
      expT = exp(scoresT/8 + mask_j)   (one ACT op; mask is per-partition bias)
      ctxT[d, i] += [v_h | 1]^T-weighted expT           (fused denominator row)
  Device emits unnormalized ctxT + denom rows [8*65, 2048]; the host divides and
  transposes into [B, S, H].

Two Trainium2-specific tricks matter here:
  * Changing the matmul contraction size (K) between back-to-back matmuls costs
    ~1.6us in PE reconfiguration, so every matmul keeps K=128: Q is stored
    per-head zero-padded to 128 partitions (the other head's K rows hit zeros),
    while KT stays packed two heads per tile.
  * All PSUM lives in one pool of 4 [128, 1024] tags: QKV passes use tile
    halves as 8 accumulators, attention ping-pongs scores on tags 0/1 and ctx
    on tags 2/3 -- no pool-transition barrier or head-boundary PE stalls.

Matmuls run as float32r (full-rate fp32 with hardware rounding, ~2e-4 rel err).
"""

import os
import sys

if "/opt/trn_rl_repo" not in sys.path:
    sys.path.insert(0, "/opt/trn_rl_repo")

import numpy as np

_KERNEL_DIR = os.path.dirname(os.path.abspath(__file__))

B, S, H = 4, 2048, 1024
NH, HD = 16, 64
HPC = 8          # heads per core
CH = HPC * HD    # 512 output channels per core
CT = H // 128    # 8 contraction tiles
JT = CH // 128   # 4 channel tiles per core
ST = S // 128    # 16 token tiles
VW = HD + 1      # 65: v columns + fused ones column

_CACHE = {}


def _build():
    import concourse.bass as bass  # noqa: F401  (registers engine methods)
    import concourse.mybir as mybir
    import concourse.tile as tile
    from concourse import bacc

    F32 = mybir.dt.float32
    F32R = mybir.dt.float32r

    nc = bacc.Bacc("TRN2", target_bir_lowering=False, debug=True)

    xt = nc.dram_tensor("xt", [H, S], F32, kind="ExternalInput")        # x_b^T
    wq_t = nc.dram_tensor("wq_t", [H, CH], F32, kind="ExternalInput")   # wq_c^T
    wk_t = nc.dram_tensor("wk_t", [H, CH], F32, kind="ExternalInput")
    wv_t = nc.dram_tensor("wv_t", [H, CH], F32, kind="ExternalInput")
    bq = nc.dram_tensor("bq", [CH], F32, kind="ExternalInput")
    bk = nc.dram_tensor("bk", [CH], F32, kind="ExternalInput")
    bv = nc.dram_tensor("bv", [CH], F32, kind="ExternalInput")
    mask = nc.dram_tensor("mask", [S], F32, kind="ExternalInput")
    ones = nc.dram_tensor("ones", [512], F32, kind="ExternalInput")
    # unnormalized ctxT + denominator rows, 65 rows per head
    out = nc.dram_tensor("out", [VW * HPC, S], F32, kind="ExternalOutput")

    with tile.TileContext(nc) as tc, nc.allow_low_precision(reason="fp32r attention"):
        from contextlib import ExitStack

        with ExitStack() as outer:
            persist = outer.enter_context(tc.tile_pool(name="persist", bufs=1))
            ppool = outer.enter_context(tc.tile_pool(name="pp", bufs=1, space="PSUM"))

            # Persistent SBUF tensors
            # Q per head, zero-padded to 128 partitions (head h lives in its own
            # partition range po:po+64; the other 64 rows are zeros).
            qp_sb = [persist.tile([128, S], F32R, tag=f"qp{h}", name=f"qp{h}")
                     for h in range(HPC)]
            kt_sb = [persist.tile([128, S], F32R, tag=f"kt{j}", name=f"kt{j}")
                     for j in range(JT)]
            v_sb = persist.tile([128, ST, VW * HPC], F32R, tag="v")
            mask_sb = persist.tile([128, ST], F32, tag="mask")
            bqp = persist.tile([128, JT], F32, tag="bqp")
            bkp = persist.tile([128, JT], F32, tag="bkp")
            bv_bc = persist.tile([128, CH], F32, tag="bv_bc")
            ones8 = persist.tile([128, HPC], F32R, tag="ones8")
            zcol = persist.tile([128, 1], F32, tag="zcol")

            nc.sync.dma_start(out=mask_sb, in_=mask.rearrange("(t p) -> p t", p=128))
            nc.sync.dma_start(out=bqp, in_=bq.rearrange("(j p) -> p j", p=128))
            nc.sync.dma_start(out=bkp, in_=bk.rearrange("(j p) -> p j", p=128))
            nc.sync.dma_start(
                out=bv_bc,
                in_=bass.AP(tensor=bv, offset=0, ap=[[0, 128], [1, CH]]))
            nc.sync.dma_start(
                out=ones8,
                in_=bass.AP(tensor=ones.bitcast(F32R), offset=0,
                            ap=[[0, 128], [1, HPC]]))
            # ones columns of v (position 64 of each head block, every token tile)
            v4 = v_sb.rearrange("p t (h e) -> p t h e", e=VW)
            for t in range(ST):
                nc.vector.tensor_copy(v4[:, t, :, HD], ones8)
            # zero the unused partition half of each padded-Q tile
            nc.vector.memset(zcol, 0.0)
            for h in range(HPC):
                zo = 64 if h % 2 == 0 else 0      # rows NOT owned by head h
                zsrc = zcol[zo:zo + 64, 0:1]
                zbcast = bass.AP(tensor=zsrc.tensor, offset=zsrc.offset,
                                 ap=[zsrc.ap[0], [0, S]])
                nc.vector.tensor_copy(qp_sb[h][zo:zo + 64, :], zbcast)

            # ---------------- Phase 1: QKV projections ----------------
            with ExitStack() as ph1:
                wpool = ph1.enter_context(tc.tile_pool(name="w", bufs=1))
                xqpool = ph1.enter_context(tc.tile_pool(name="xq", bufs=10))
                xpool = ph1.enter_context(tc.tile_pool(name="x", bufs=4))

                wq_r = wq_t.rearrange("(c p) j -> c p j", p=128).bitcast(F32R)
                wk_r = wk_t.rearrange("(c p) j -> c p j", p=128).bitcast(F32R)
                wv_r = wv_t.rearrange("(c p) j -> c p j", p=128).bitcast(F32R)
                xt_r = xt.rearrange("(c p) s -> c p s", p=128).bitcast(F32R)

                # stage the full first quarter: per ct interleave x, wq, wk
                # DMAs so the first pass never runs dry; wv is deferred until
                # after the QK passes (the V pass runs last).
                x_first = []
                wq_sb, wk_sb, wv_sb = [], [], []
                for ct in range(CT):
                    x_t = xqpool.tile([128, 512], F32R, tag="xq", name=f"xqk0{ct}")
                    nc.sync.dma_start(out=x_t, in_=xt_r[ct, :, 0:512])
                    x_first.append(x_t)
                    for lst, srct, nm in ((wq_sb, wq_r, "wq"), (wk_sb, wk_r, "wk")):
                        w = wpool.tile([128, CH], F32R, tag=f"{nm}{ct}",
                                       name=f"{nm}{ct}")
                        nc.sync.dma_start(out=w, in_=srct[ct])
                        lst.append(w)

                # Combined Q+K pass over query-range quarters (x streamed once).
                # PSUM tag t{j} holds Q_j in columns 0:512 and K_j in 512:1024.
                for sq in range(4):
                    pqk = [ppool.tile([128, 1024], F32, tag=f"t{j}",
                                      name=f"pqk{sq}{j}")
                           for j in range(JT)]
                    for ct in range(CT):
                        if sq == 0:
                            x_t = x_first[ct]
                        else:
                            x_t = xqpool.tile([128, 512], F32R, tag="xq",
                                              name=f"xqk{sq}_{ct}")
                            nc.sync.dma_start(
                                out=x_t,
                                in_=xt_r[ct, :, sq * 512:(sq + 1) * 512])
                        for j in range(JT):
                            nc.tensor.matmul(
                                pqk[j][:, 0:512],
                                lhsT=wq_sb[ct][:, j * 128:(j + 1) * 128],
                                rhs=x_t,
                                start=(ct == 0), stop=(ct == CT - 1))
                        for j in range(JT):
                            nc.tensor.matmul(
                                pqk[j][:, 512:1024],
                                lhsT=wk_sb[ct][:, j * 128:(j + 1) * 128],
                                rhs=x_t,
                                start=(ct == 0), stop=(ct == CT - 1))
                    for j in range(JT):
                        # drain each tag via three engines-worth of copies:
                        # q head-even on ACT, q head-odd + k on DVE
                        h0, h1 = 2 * j, 2 * j + 1
                        nc.scalar.activation(
                            qp_sb[h0][0:64, sq * 512:(sq + 1) * 512],
                            pqk[j][0:64, 0:512],
                            mybir.ActivationFunctionType.Identity,
                            bias=bqp[0:64, j:j + 1], scale=1.0)
                        nc.vector.tensor_scalar_add(
                            qp_sb[h1][64:128, sq * 512:(sq + 1) * 512],
                            pqk[j][64:128, 0:512],
                            bqp[64:128, j:j + 1])
                        nc.vector.tensor_scalar_add(
                            kt_sb[j][:, sq * 512:(sq + 1) * 512],
                            pqk[j][:, 512:1024],
                            bkp[:, j:j + 1])

                # V pass: tokens on psum partitions (x streamed a second time).
                for ct in range(CT):
                    w = wpool.tile([128, CH], F32R, tag=f"wq{ct}", name=f"wv{ct}")
                    nc.sync.dma_start(out=w, in_=wv_r[ct])
                    wv_sb.append(w)
                for sh in range(2):
                    pv = [ppool.tile([128, 1024], F32, tag=f"t{j}",
                                     name=f"pv{sh}{j}")
                          for j in range(JT)]
                    for ct in range(CT):
                        x_t = xpool.tile([128, 1024], F32R, tag="x",
                                         name=f"xv{sh}{ct}")
                        nc.sync.dma_start(
                            out=x_t, in_=xt_r[ct, :, sh * 1024:(sh + 1) * 1024])
                        for st in range(8):
                            nc.tensor.matmul(
                                pv[st // 2][:, (st % 2) * 512:(st % 2 + 1) * 512],
                                lhsT=x_t[:, st * 128:(st + 1) * 128],
                                rhs=wv_sb[ct],
                                start=(ct == 0), stop=(ct == CT - 1))
                    for st in range(8):
                        sl = pv[st // 2][:, (st % 2) * 512:(st % 2 + 1) * 512]
                        for h in range(HPC):
                            nc.vector.tensor_add(
                                v_sb[:, sh * 8 + st, h * VW:h * VW + HD],
                                sl[:, h * HD:(h + 1) * HD],
                                bv_bc[:, h * HD:(h + 1) * HD])

            # ---------------- Phase 2: attention ----------------
            with ExitStack() as ph2:
                epool = ph2.enter_context(tc.tile_pool(name="ep", bufs=8))
                opool = ph2.enter_context(tc.tile_pool(name="op", bufs=3))

                for h in range(HPC):
                    qi = h // 2
                    for ih in range(2):
                        blk = h * 2 + ih
                        i0 = ih * 1024
                        ctx_ps = ppool.tile([VW, 1024], F32, tag=f"t{2 + blk % 2}",
                                            name=f"ctx{blk}")
                        for st in range(ST):
                            s_ps = ppool.tile([128, 1024], F32, tag=f"t{st % 2}",
                                              name=f"sc{blk}_{st}")
                            for q in range(2):
                                nc.tensor.matmul(
                                    s_ps[:, q * 512:(q + 1) * 512],
                                    lhsT=kt_sb[qi][:, st * 128:(st + 1) * 128],
                                    rhs=qp_sb[h][:, i0 + q * 512:i0 + (q + 1) * 512],
                                    start=True, stop=True)
                            e_sb = epool.tile([128, 1024], F32R, tag="e",
                                              name=f"e{blk}_{st}")
                            nc.scalar.activation(
                                e_sb, s_ps,
                                mybir.ActivationFunctionType.Exp,
                                bias=mask_sb[:, st:st + 1], scale=0.125)
                            for q in range(2):
                                nc.tensor.matmul(
                                    ctx_ps[:, q * 512:(q + 1) * 512],
                                    lhsT=v_sb[:, st, h * VW:(h + 1) * VW],
                                    rhs=e_sb[:, q * 512:(q + 1) * 512],
                                    start=(st == 0), stop=(st == ST - 1))
                        o_sb = opool.tile([VW, 1024], F32, tag="o", name=f"o{blk}")
                        nc.vector.tensor_copy(o_sb, ctx_ps)
                        nc.sync.dma_start(
                            out=out[h * VW:(h + 1) * VW, i0:i0 + 1024], in_=o_sb)

    nc.compile()
    return nc


def _get_nc():
    if "nc" not in _CACHE:
        _CACHE["nc"] = _build()
    return _CACHE["nc"]


def _in_maps(hidden_states, attention_mask, wq, bq, wk, bk, wv, bv):
    ones = np.ones(512, np.float32)
    maps = []
    for c in range(8):
        b, g = c // 2, c % 2
        ch0 = g * CH
        maps.append({
            "xt": np.ascontiguousarray(hidden_states[b].T),
            "wq_t": np.ascontiguousarray(wq[ch0:ch0 + CH, :].T),
            "wk_t": np.ascontiguousarray(wk[ch0:ch0 + CH, :].T),
            "wv_t": np.ascontiguousarray(wv[ch0:ch0 + CH, :].T),
            "bq": np.ascontiguousarray(bq[ch0:ch0 + CH]),
            "bk": np.ascontiguousarray(bk[ch0:ch0 + CH]),
            "bv": np.ascontiguousarray(bv[ch0:ch0 + CH]),
            "mask": np.ascontiguousarray(attention_mask[b, 0, 0, :]),
            "ones": ones,
        })
    return maps


def _gather(results):
    full = np.empty((B, S, H), np.float32)
    for c in range(8):
        b, g = c // 2, c % 2
        o = results[c]["out"].reshape(HPC, VW, S)
        ctx = o[:, :HD, :] / o[:, HD:HD + 1, :]        # normalize by denom row
        # [h, d, s] -> [s, h*d]
        full[b, :, g * CH:(g + 1) * CH] = ctx.reshape(CH, S).T
    return full


def _run(in_maps, trace=False):
    from concourse.bass_utils import run_bass_kernel_spmd

    nc = _get_nc()
    return run_bass_kernel_spmd(nc, in_maps, list(range(8)), trace=trace)


def _run_results(in_maps):
    """Run on hardware; on a wedged-device error retry in fresh subprocesses
    (the PJRT client cannot recover an unrecoverable exec unit in-process)."""
    try:
        return _run(in_maps).results
    except Exception:
        pass
    import pickle
    import subprocess
    import tempfile

    last = None
    for _ in range(3):
        try:
            with tempfile.TemporaryDirectory() as td:
                fin = os.path.join(td, "in.pkl")
                fout = os.path.join(td, "out.pkl")
                with open(fin, "wb") as f:
                    pickle.dump(in_maps, f)
                code = (
                    "import pickle, sys\n"
                    f"sys.path.insert(0, {_KERNEL_DIR!r})\n"
                    "import kernel\n"
                    f"maps = pickle.load(open({fin!r}, 'rb'))\n"
                    "res = kernel._run(maps)\n"
                    f"pickle.dump(res.results, open({fout!r}, 'wb'))\n"
                )
                subprocess.run([sys.executable, "-c", code], check=True,
                               timeout=1800)
                with open(fout, "rb") as f:
                    return pickle.load(f)
        except Exception as e:
            last = e
    raise last


def kernel(hidden_states, attention_mask, wq, bq, wk, bk, wv, bv):
    args = [np.asarray(a, np.float32) for a in
            (hidden_states, attention_mask, wq, bq, wk, bk, wv, bv)]
    return _gather(_run_results(_in_maps(*args)))


def kernel_profiled(hidden_states, attention_mask, wq, bq, wk, bk, wv, bv):
    """Like kernel() but with NTFF tracing; returns (output, exec_time_ns)."""
    args = [np.asarray(a, np.float32) for a in
            (hidden_states, attention_mask, wq, bq, wk, bk, wv, bv)]
    res = _run(_in_maps(*args), trace=True)
    return _gather(res.results), res.exec_time_ns

